# revision 27
# baseline (speedup 1.0000x reference)
"""MultiHeadAttention forward on 8 Trainium2 NeuronCores (Bass/Tile), v2.

Problem (hardcoded): B=2, S=2048, D=1024, H=16, DK=64, causal mask.

Sharding: data-parallel over batch (2) x tensor-parallel over heads
(4 heads per core).  core = 4*b + g handles batch b, heads [4g, 4g+4).
Host sums the 4 partial O outputs per batch and adds bo.

v2 restructure vs baseline:
  - Global software pipeline: next-chunk QKV-projection and prev-chunk
    O-projection matmuls are emitted as "filler" units interleaved into
    the attention j-loop, so the PE never starves while the Activation
    engine paces exp().
  - Causal handling: scores/exp/AV for diagonal k-tiles are
    range-restricted to valid queries; only the 128x128 diagonal block
    gets a tri-mask multiply (no DVE column zeroing).
  - Normalize chain: partition_broadcast reads the reciprocal row at
    partition 64 directly (no 1-partition staging DMA).
  - QT/KT in bf16; output partials in bf16 (host sums in f32).
  - DMA spread: wq/wk + k input on SP queue, q/v inputs + wv/wo + out
    on Pool queue, biases/tri/wo_lo on Act queue.
  - PSUM: sc 2x[128,1024] (4 banks) + ctx 2x[65,512] (2) + wk 2x[128,512]
    (2) = 8 banks.
  - Tail: last chunk's O-projection split into waves (pair-1 K=128, h0
    K=64 from cx, h1 K=64 from tmp via lane-shifted wo_lo) so no cx
    merge DMA sits on the critical path.
"""

import os
import sys

sys.path.insert(0, "/opt/trn_rl_repo")

import numpy as np

B, S, D, H = 2, 2048, 1024, 16
DK = D // H          # 64
NCORES = 8
G = 4                # heads per core
DG = G * DK          # 256 output dims per core
SC = 512             # seq chunk
NCH = S // SC        # 4
KI = D // 128        # 8 contraction chunks
NPAIR = 2            # head pairs per core
NST = SC // 128      # 4 seq 128-tiles per chunk

_programs = {}
LAST_RESULT = None


def _build_program(causal: bool, reps: int = 1):
    import concourse.bass as bass
    import concourse.tile as tile
    import concourse.mybir as mybir
    from concourse import bacc
    from contextlib import ExitStack
    from collections import deque

    F32 = mybir.dt.float32
    F32R = mybir.dt.float32r
    BF16 = mybir.dt.bfloat16
    AF = mybir.ActivationFunctionType

    nc = bacc.Bacc()
    # inputs are PRE-TILED host-side into their exact SBUF layouts so every
    # load is a flat contiguous DMA (128 descriptors x 4-8KB, no gather):
    #   qT/kT/vT: [128, NCH*8*SC]; chunk c occupies cols [c*8*SC,(c+1)*8*SC)
    #   with the ki-th 128-row d-block of that chunk at sub-cols
    #   [ki*SC,(ki+1)*SC) (h-major: ki = 4*h + a).
    qT_d = nc.dram_tensor("qT", [128, NCH * KI * SC], BF16, kind="ExternalInput")
    kT_d = nc.dram_tensor("kT", [128, NCH * KI * SC], BF16, kind="ExternalInput")
    vT_d = nc.dram_tensor("vT", [128, NCH * KI * SC], BF16, kind="ExternalInput")
    wq_d = nc.dram_tensor("wqT", [128, KI * DG], BF16, kind="ExternalInput")
    wk_d = nc.dram_tensor("wkT", [128, KI * DG], BF16, kind="ExternalInput")
    wv_d = nc.dram_tensor("wvT", [128, KI * DG], BF16, kind="ExternalInput")
    wo_d = nc.dram_tensor("woT", [128, 2 * D], BF16, kind="ExternalInput")
    bq_d = nc.dram_tensor("bq2", [128, NPAIR], F32, kind="ExternalInput")
    bk_d = nc.dram_tensor("bk2", [128, NPAIR], F32, kind="ExternalInput")
    bv_d = nc.dram_tensor("bvb", [128, DG], F32, kind="ExternalInput")
    tri_d = nc.dram_tensor("tri", [128, 128], BF16, kind="ExternalInput")
    out_d = nc.dram_tensor("out", [S, D], BF16, kind="ExternalOutput")

    with ExitStack() as ctx:
        tc = ctx.enter_context(tile.TileContext(nc))
        cpool = ctx.enter_context(tc.tile_pool(name="const", bufs=1))
        inpool = ctx.enter_context(tc.tile_pool(name="ins", bufs=5))
        qtpool = ctx.enter_context(tc.tile_pool(name="qt", bufs=1))
        ktpool = ctx.enter_context(tc.tile_pool(name="kt", bufs=1))
        vpool = ctx.enter_context(tc.tile_pool(name="v", bufs=1))
        cxpool = ctx.enter_context(tc.tile_pool(name="cx", bufs=1))
        prpool = ctx.enter_context(tc.tile_pool(name="probs", bufs=4))
        rcpool = ctx.enter_context(tc.tile_pool(name="recip", bufs=3))
        bcpool = ctx.enter_context(tc.tile_pool(name="bc", bufs=4))
        tmpool = ctx.enter_context(tc.tile_pool(name="tmp", bufs=3))
        outpool = ctx.enter_context(tc.tile_pool(name="osb", bufs=6))
        psS = ctx.enter_context(tc.tile_pool(name="psS", bufs=2, space="PSUM"))
        psC = ctx.enter_context(tc.tile_pool(name="psC", bufs=2, space="PSUM"))
        psW = ctx.enter_context(tc.tile_pool(name="psW", bufs=2, space="PSUM"))

        QT = {}
        KT = {}
        V = {}
        CX = {}
        INS = {}

        fillers = deque()   # hard: QK-proj of next chunk; flushed at boundary
        soft = deque()      # V-proj of next chunk + O-proj of prev chunk

        def drain(n=None, include_soft=True):
            k = (len(fillers) + (len(soft) if include_soft else 0)) if n is None else n
            for _ in range(k):
                if fillers:
                    fillers.popleft()()
                elif include_soft and soft:
                    soft.popleft()()
                else:
                    break

        # ---------- filler-unit builders (each closure = ~1 PE matmul) ----

        def qk_proj_fillers(w_sb, b_sb, in_sb, t, dst, c):
            """8 matmuls accumulating one [128,512] psum + bias-add -> bf16."""
            state = {}

            def mk(ki):
                def f():
                    if ki == 0:
                        state["ps"] = psW.tile(
                            [128, SC], F32, tag="wk", name="wkps"
                        )
                    nc.tensor.matmul(
                        state["ps"][:],
                        w_sb[:, (t * KI + ki) * 128 : (t * KI + ki + 1) * 128],
                        in_sb[:, ki * SC : (ki + 1) * SC],
                        start=(ki == 0),
                        stop=(ki == KI - 1),
                    )
                    if ki == KI - 1:
                        pool = qtpool if dst is QT else ktpool
                        tl = pool.tile(
                            [128, SC], BF16,
                            tag=f"{'q' if dst is QT else 'k'}{t}{c}",
                            name="qkt",
                        )
                        nc.vector.tensor_scalar_add(
                            tl[:], state["ps"][:], b_sb[:, t : t + 1]
                        )
                        dst[(t, c)] = tl

                return f

            return [mk(ki) for ki in range(KI)]

        def v_proj_fillers(wv_sb, bv_sb, in_sb, st, c):
            """8 matmuls -> [128,256] psum + assemble V[j] with ones cols."""
            j = c * NST + st
            state = {}

            def mk(ki):
                def f():
                    if ki == 0:
                        state["ps"] = psW.tile(
                            [128, SC], F32, tag="wk", name="wkps"
                        )
                    nc.tensor.matmul(
                        state["ps"][:, 0:DG],
                        in_sb[
                            :,
                            ki * SC + st * 128 : ki * SC + (st + 1) * 128,
                        ],
                        wv_sb[:, ki * DG : (ki + 1) * DG],
                        start=(ki == 0),
                        stop=(ki == KI - 1),
                    )
                    if ki == KI - 1:
                        vt = vpool.tile(
                            [128, G * 65], BF16, tag=f"v{j}", name="vt"
                        )
                        nc.vector.tensor_scalar(
                            vt[:].rearrange("p (h x) -> p h x", x=65)[:, :, 0:1],
                            bv_sb[:].rearrange("p (a b) -> p a b", b=1)[:, 0:G, :],
                            0.0,
                            1.0,
                            mybir.AluOpType.mult,
                            mybir.AluOpType.add,
                        )
                        nc.vector.tensor_add(
                            vt[:].rearrange("p (h x) -> p h x", x=65)[:, :, 1:65],
                            state["ps"][:, 0:DG].rearrange(
                                "p (h x) -> p h x", x=64
                            ),
                            bv_sb[:].rearrange("p (h x) -> p h x", x=64),
                        )
                        V[j] = vt

                return f

            return [mk(ki) for ki in range(KI)]

        def o_proj_fillers(c):
            """Partial O-projection for chunk c: per (st, half) one 2-matmul
            unit + psum->osb bf16 copy; DMA per st row."""
            units = []
            state = {}

            def mk(st, n):
                def f():
                    if n == 0:
                        state[st] = outpool.tile(
                            [128, D], BF16, tag="osb", name="osb"
                        )
                    ps = psW.tile([128, SC], F32, tag="wk", name="wkps")
                    for p in (1, 0):
                        nc.tensor.matmul(
                            ps[:],
                            CX[(p, c)][:, st * 128 : (st + 1) * 128],
                            wo_sb[:, p * D + n * SC : p * D + (n + 1) * SC],
                            start=(p == 1),
                            stop=(p == 0),
                        )
                    osb = state[st]
                    if n == 0:
                        nc.vector.tensor_copy(osb[:, 0:SC], ps[:])
                    else:
                        nc.vector.tensor_copy(osb[:, SC : 2 * SC], ps[:])
                        r0 = (c * NST + st) * 128
                        # sync queue, not gpsimd: keeps the gpsimd queue
                        # free for the latency-critical partition_broadcasts
                        nc.sync.dma_start(out_d[r0 : r0 + 128, :], osb[:])

                return f

            for st in range(NST):
                for n in range(2):
                    units.append(mk(st, n))
            return units

        def o_proj_tail(c):
            """Tail O-projection: pair-1 contractions first (its cx merges
            early), then pair-0 contractions + copies/DMAs spread over
            DVE/Act + SP/Pool/Act queues.  psum spread over psW/psC/psS."""
            pss = {}
            for n in range(2):
                pss[(0, n)] = psW.tile([128, SC], F32, tag="wk", name="otps")
                pss[(1, n)] = psC.tile([128, SC], F32, tag="ctx", name="otps")
            for st in (2, 3):
                big = psS.tile([128, 2 * SC], F32, tag="sc", name="otps")
                pss[(st, 0)] = big[:, 0:SC]
                pss[(st, 1)] = big[:, SC : 2 * SC]
            # wave A: pair-1 contractions for sts not on psC (ready early)
            for st in (0, 2, 3):
                for n in range(2):
                    nc.tensor.matmul(
                        pss[(st, n)],
                        CX[(1, c)][:, st * 128 : (st + 1) * 128],
                        wo_sb[:, D + n * SC : D + (n + 1) * SC],
                        start=True,
                        stop=False,
                        skip_group_check=True,
                    )
            # st1 on psC slots (free once the normalize consumed ctx0/ctx1)
            for n in range(2):
                nc.tensor.matmul(
                    pss[(1, n)],
                    CX[(1, c)][:, 128:256],
                    wo_sb[:, D + n * SC : D + (n + 1) * SC],
                    start=True,
                    stop=False,
                    skip_group_check=True,
                )
            # wave B: pair-0 contractions on merged cx + copies (DVE/Act
            # alternating) + bf16 out DMAs on sync/gpsimd
            ei = 0
            copies = (nc.vector, nc.scalar, nc.vector, nc.scalar,
                      nc.vector, nc.scalar, nc.vector, nc.scalar)
            dmas = (nc.sync, nc.gpsimd, nc.sync, nc.gpsimd,
                    nc.sync, nc.gpsimd, nc.sync, nc.gpsimd)
            osbs = {}
            for st in (0, 2, 3, 1):
                osbs[st] = outpool.tile([128, D], BF16, tag="osb", name="osb")
            for st in (0, 2, 3, 1):
                osb = osbs[st]
                r0 = (c * NST + st) * 128
                for n in range(2):
                    nc.tensor.matmul(
                        pss[(st, n)],
                        CX[(0, c)][:, st * 128 : (st + 1) * 128],
                        wo_sb[:, n * SC : (n + 1) * SC],
                        start=False,
                        stop=True,
                        skip_group_check=True,
                    )
                    ceng = copies[ei]
                    if ceng is nc.scalar:
                        ceng.copy(osb[:, n * SC : (n + 1) * SC], pss[(st, n)])
                    else:
                        ceng.tensor_copy(
                            osb[:, n * SC : (n + 1) * SC], pss[(st, n)]
                        )
                    dmas[ei].dma_start(
                        out_d[r0 : r0 + 128, n * SC : (n + 1) * SC],
                        osb[:, n * SC : (n + 1) * SC],
                    )
                    ei += 1

        def stream_in(c, tag, t_d, eng):
            t_sb = inpool.tile(
                [128, KI * SC], BF16, tag=tag, name="t_sb", bufs=3
            )
            eng.dma_start(
                t_sb[:], t_d[:, c * KI * SC : (c + 1) * KI * SC]
            )
            return t_sb

        def emit_input_dmas(c):
            # all on sync: a gpsimd dma_start unloads the
            # partition_broadcast library (MODIFY_POOL_CONFIG) and the
            # reload waits on the gpsimd DMA queue drain — never mix them
            qin = stream_in(c, "qin", qT_d, nc.sync)
            kin = stream_in(c, "kin", kT_d, nc.sync)
            vin = stream_in(c, "vin", vT_d, nc.sync)
            return qin, kin, vin

        # ---------- startup -------------------------------------------------

        if reps > 1:
            ctx.enter_context(
                tc.For_i(
                    0,
                    reps,
                    1,
                    hint_engines=(
                        mybir.EngineType.PE,
                        mybir.EngineType.Activation,
                        mybir.EngineType.DVE,
                        mybir.EngineType.SP,
                        mybir.EngineType.Pool,
                    ),
                )
            )

        wq_sb = cpool.tile([128, KI * DG], BF16, tag="wq")
        wk_sb = cpool.tile([128, KI * DG], BF16, tag="wk")
        wv_sb = cpool.tile([128, KI * DG], BF16, tag="wv")
        wo_sb = cpool.tile([128, 2 * D], BF16, tag="wo")
        # wq/wk are t-major host-side; pair t=1 is projected first, so its
        # half loads first
        for lo, hi in ((1024, 1536), (1536, 2048), (0, 1024)):
            nc.sync.dma_start(wq_sb[:, lo:hi], wq_d[:, lo:hi])
        bq_sb = cpool.tile([128, NPAIR], F32, tag="bq")
        bk_sb = cpool.tile([128, NPAIR], F32, tag="bk")
        bv_sb = cpool.tile([128, DG], F32, tag="bv")
        tri_sb = cpool.tile([128, 128], BF16, tag="tri")
        nc.scalar.dma_start(bq_sb[:], bq_d[:])
        nc.scalar.dma_start(bk_sb[:], bk_d[:])
        nc.scalar.dma_start(bv_sb[:], bv_d[:])
        nc.scalar.dma_start(tri_sb[:], tri_d[:])

        # chunk-0 inputs, split into pieces so the first projection fillers
        # unblock as soon as their ki-blocks land
        qin = inpool.tile([128, KI * SC], BF16, tag="qin", name="qin", bufs=3)
        for lo, hi in ((0, 1), (1, 2), (2, 4), (4, 6), (6, 8)):
            nc.gpsimd.dma_start(
                qin[:, lo * SC : hi * SC], qT_d[:, lo * SC : hi * SC]
            )
        nc.sync.dma_start(wk_sb[:, 1024:2048], wk_d[:, 1024:2048])
        kin = inpool.tile([128, KI * SC], BF16, tag="kin", name="kin", bufs=3)
        for (lo, hi), eng in (((0, 4), nc.sync), ((4, 8), nc.gpsimd)):
            eng.dma_start(
                kin[:, lo * SC : hi * SC], kT_d[:, lo * SC : hi * SC]
            )
        nc.sync.dma_start(wk_sb[:, 0:1024], wk_d[:, 0:1024])
        nc.gpsimd.dma_start(wv_sb[:], wv_d[:])
        # vin/wo ride the Act hardware-DGE queue: it is idle before the
        # first exps, giving a third parallel transfer queue at startup
        vin = stream_in(0, "vin", vT_d, nc.scalar)
        nc.scalar.dma_start(wo_sb[:], wo_d[:])

        # chunk-0: pair-1 Q/K projections drain now (attention needs them);
        # V + pair-0 projections become in-loop fillers.  V goes FIRST in
        # the hard queue: pair-1's AVs need V[0..3] right away, and they
        # must not sit behind chunk-1 projections that are gated on
        # chunk-1 input DMAs.
        fillers.extend(qk_proj_fillers(wq_sb, bq_sb, qin, 1, QT, 0))
        fillers.extend(qk_proj_fillers(wk_sb, bk_sb, kin, 1, KT, 0))
        drain(include_soft=False)
        for st in range(NST):
            fillers.extend(v_proj_fillers(wv_sb, bv_sb, vin, st, 0))
        fillers.extend(qk_proj_fillers(wq_sb, bq_sb, qin, 0, QT, 0))
        fillers.extend(qk_proj_fillers(wk_sb, bk_sb, kin, 0, KT, 0))
        if not causal:
            # full-width attention reads every chunk's K/V from chunk 0 on:
            # project everything up front (correctness over overlap)
            drain()
            for cc in range(1, NCH):
                qin, kin, vin = emit_input_dmas(cc)
                for t in (1, 0):
                    fillers.extend(
                        qk_proj_fillers(wq_sb, bq_sb, qin, t, QT, cc)
                    )
                    fillers.extend(
                        qk_proj_fillers(wk_sb, bk_sb, kin, t, KT, cc)
                    )
                for st in range(NST):
                    fillers.extend(v_proj_fillers(wv_sb, bv_sb, vin, st, cc))
                drain()

        # ---------- main loop over chunks ---------------------------------

        for c in range(NCH):
            # hard queue: QK-proj of c+1 (first-processed pair first);
            # soft queue: V-proj of c+1, then O-proj of c-1.
            # inputs are prefetched TWO chunks ahead (ring bufs=3) so the
            # c+1 projection fillers never stall the attention stream on
            # input arrival
            if causal and c == 0:
                INS[1] = emit_input_dmas(1)
                if NCH > 2:
                    INS[2] = emit_input_dmas(2)
            elif causal and c + 2 < NCH:
                INS[c + 2] = emit_input_dmas(c + 2)
            if causal and c + 1 < NCH:
                qin, kin, vin = INS[c + 1]
                for t in (1, 0):
                    fillers.extend(
                        qk_proj_fillers(wq_sb, bq_sb, qin, t, QT, c + 1)
                    )
                    fillers.extend(
                        qk_proj_fillers(wk_sb, bk_sb, kin, t, KT, c + 1)
                    )
                for st in range(NST):
                    soft.extend(v_proj_fillers(wv_sb, bv_sb, vin, st, c + 1))
            if c > 0:
                soft.extend(o_proj_fillers(c - 1))

            # attention j-loop for chunk c, per head pair
            for p in (1, 0):
                while ((p, c) not in QT or (p, c) not in KT) and (
                    fillers or soft
                ):
                    drain(1)
                njt = NST * (c + 1) if causal else NST * NCH
                ctx0 = psC.tile([65, SC], F32, tag="ctx")
                ctx1 = psC.tile([65, SC], F32, tag="ctx")
                h0, h1 = 2 * p, 2 * p + 1
                pending = None

                def emit_av(j, probs, first, last, q0):
                    nc.tensor.matmul(
                        ctx0[:, q0:SC],
                        V[j][:, 65 * h0 : 65 * h0 + 65],
                        probs[:, q0:SC],
                        start=first,
                        stop=last,
                        skip_group_check=True,
                    )
                    nc.tensor.matmul(
                        ctx1[:, q0:SC],
                        V[j][:, 65 * h1 : 65 * h1 + 65],
                        probs[:, SC + q0 : 2 * SC],
                        start=first,
                        stop=last,
                        skip_group_check=True,
                    )

                for j in range(njt):
                    jc, jt = divmod(j, NST)
                    diag = causal and jc == c
                    m = jt if diag else 0
                    q0 = 128 * m  # first valid query col in this chunk
                    scp = psS.tile([128, 2 * SC], F32, tag="sc")
                    nc.tensor.matmul(
                        scp[:, q0:SC],
                        KT[(p, jc)][0:64, jt * 128 : (jt + 1) * 128],
                        QT[(p, c)][0:64, q0:SC],
                        start=True,
                        stop=True,
                        tile_position=(0, 0),
                    )
                    nc.tensor.matmul(
                        scp[:, SC + q0 : 2 * SC],
                        KT[(p, jc)][64:128, jt * 128 : (jt + 1) * 128],
                        QT[(p, c)][64:128, q0:SC],
                        start=True,
                        stop=True,
                        tile_position=(64, 0),
                    )
                    probs = prpool.tile([128, 2 * SC], BF16, tag="probs")
                    if m == 0:
                        nc.scalar.activation(
                            probs[:], scp[:], AF.Exp, scale=0.125
                        )
                    else:
                        nc.scalar.activation(
                            probs[:, q0:SC], scp[:, q0:SC], AF.Exp, scale=0.125
                        )
                        nc.scalar.activation(
                            probs[:, SC + q0 : 2 * SC],
                            scp[:, SC + q0 : 2 * SC],
                            AF.Exp,
                            scale=0.125,
                        )
                    if diag:
                        # tri-mask the 128x128 diagonal block of both heads
                        for off in (0, SC):
                            lo = off + q0
                            nc.vector.tensor_mul(
                                probs[:, lo : lo + 128],
                                probs[:, lo : lo + 128],
                                tri_sb[:],
                            )
                    if pending is not None:
                        while pending[0] not in V and (fillers or soft):
                            drain(1)
                        emit_av(*pending)
                        drain(1)
                    pending = (j, probs, j == 0, j == njt - 1, q0)
                while pending[0] not in V and (fillers or soft):
                    drain(1)
                jL, probsL, firstL, lastL, q0L = pending
                nc.tensor.matmul(
                    ctx0[:, q0L:SC],
                    V[jL][:, 65 * h0 : 65 * h0 + 65],
                    probsL[:, q0L:SC],
                    start=firstL,
                    stop=lastL,
                    skip_group_check=True,
                )
                # ---- eager normalize v3: copy ctx psum -> SBUF raw f32
                # immediately (the copy is the ONLY psum reader, so psC
                # frees ~0.6us after the last AV instead of after the
                # whole recip->broadcast->mul chain, unblocking the next
                # pair's AVs).  Reciprocals use the ~5x faster approx
                # custom-DVE op.  The very last pair skips the copies and
                # normalizes straight from psum: nothing is waiting on the
                # psC banks there, and the tail wants the shortest chain.
                last_pair = p == 0 and c == NCH - 1
                rec = rcpool.tile([1, 2 * SC], F32, tag="recip")
                if last_pair:
                    raw0 = ctx0
                else:
                    raw0 = tmpool.tile([65, SC], F32, tag="raw0")
                    nc.vector.tensor_copy(raw0[:], ctx0[0:65, :])
                nc.vector.reciprocal_approx_fast(
                    rec[0:1, 0:SC], raw0[0:1, :]
                )
                nc.tensor.matmul(
                    ctx1[:, q0L:SC],
                    V[jL][:, 65 * h1 : 65 * h1 + 65],
                    probsL[:, SC + q0L : 2 * SC],
                    start=firstL,
                    stop=lastL,
                    skip_group_check=True,
                )
                if last_pair:
                    raw1 = ctx1
                else:
                    # Act engine: runs parallel to the DVE raw0 copy, so
                    # both psC banks free ~one copy-time after the last AV
                    raw1 = tmpool.tile([65, SC], F32, tag="raw1")
                    nc.scalar.copy(raw1[:], ctx1[0:65, :])
                nc.vector.reciprocal_approx_fast(
                    rec[0:1, SC : 2 * SC], raw1[0:1, :]
                )
                if p == 0:
                    # flush next-chunk projections now so their DVE
                    # bias-adds queue ahead of this pair's normalize muls
                    drain(include_soft=False)

                cx = cxpool.tile([128, SC], BF16, tag=f"cx{p}{c}")
                bc0 = bcpool.tile([65, SC], F32, tag="bc")
                nc.gpsimd.partition_broadcast(bc0[:], rec[0:1, 0:SC])
                bc1 = bcpool.tile([65, SC], F32, tag="bc")
                nc.gpsimd.partition_broadcast(bc1[:], rec[0:1, SC : 2 * SC])
                cxa = tmpool.tile([65, SC], BF16, tag="cxa")
                nc.vector.tensor_mul(cxa[0:65, :], raw0[0:65, :], bc0[0:65, :])
                tmp = tmpool.tile([65, SC], BF16, tag="tmp")
                nc.vector.tensor_mul(tmp[0:65, :], raw1[0:65, :], bc1[0:65, :])
                # cx merge DMAs both on sync (pure data waits there); the
                # very last pair's h1 merge goes on Act so the tail
                # O-projection isn't queued behind anything on sync
                h1eng = nc.scalar if last_pair else nc.sync
                nc.sync.dma_start(cx[0:64, :], cxa[1:65, :])
                h1eng.dma_start(cx[64:128, :], tmp[1:65, :])
                CX[(p, c)] = cx

            # boundary: flush hard queue (QK proj of c+1); soft carries over
            drain(include_soft=False)

        drain()
        o_proj_tail(NCH - 1)

    nc.finalize()
    return nc


def get_program(causal: bool):
    if causal not in _programs:
        _programs[causal] = _build_program(causal)
    return _programs[causal]


def _tile_seq(xT):
    """[D, S] -> [128, NCH*KI*SC]: chunk-major, then 128-row d-block (ki)."""
    return np.ascontiguousarray(
        xT.reshape(KI, 128, NCH, SC).transpose(1, 2, 0, 3).reshape(128, -1)
    )


def _tile_w(wT, nblk):
    """[nblk*128, M] -> [128, nblk*M]: 128-row block i at cols [i*M,(i+1)*M)."""
    m = wT.shape[1]
    return np.ascontiguousarray(
        wT.reshape(nblk, 128, m).transpose(1, 0, 2).reshape(128, nblk * m)
    )


def _tile_w_tmaj(wT):
    """[D, DG] -> [128, 2*KI*128], t-major: (t, ki) block at
    cols [(t*KI+ki)*128, ...)."""
    return np.ascontiguousarray(
        wT.reshape(KI, 128, NPAIR, 128)
        .transpose(1, 2, 0, 3)
        .reshape(128, NPAIR * KI * 128)
    )


def _make_core_inputs(query, key, value, wq, bq, wk, bk, wv, bv, wo):
    import ml_dtypes

    bf16 = ml_dtypes.bfloat16
    f32 = np.float32
    tri = np.triu(np.ones((128, 128), f32)).astype(bf16)
    in_maps = []
    qTt = [_tile_seq(query[b].T.astype(bf16)) for b in range(B)]
    kTt = [_tile_seq(key[b].T.astype(bf16)) for b in range(B)]
    vTt = [_tile_seq(value[b].T.astype(bf16)) for b in range(B)]
    for core in range(NCORES):
        b, g = divmod(core, G)
        sl = slice(g * DG, (g + 1) * DG)
        in_maps.append(
            {
                "qT": qTt[b],
                "kT": kTt[b],
                "vT": vTt[b],
                "wqT": _tile_w_tmaj(wq[sl, :].T.astype(bf16)),
                "wkT": _tile_w_tmaj(wk[sl, :].T.astype(bf16)),
                "wvT": _tile_w(wv[sl, :].T.astype(bf16), KI),
                "woT": _tile_w(wo[:, sl].T.astype(bf16), 2),
                "bq2": np.ascontiguousarray(bq[sl].reshape(NPAIR, 128).T, f32),
                "bk2": np.ascontiguousarray(bk[sl].reshape(NPAIR, 128).T, f32),
                "bvb": np.ascontiguousarray(
                    np.broadcast_to(bv[sl], (128, DG)), f32
                ),
                "tri": tri,
            }
        )
    return in_maps


def _numpy_fallback(query, key, value, mask, wq, bq, wk, bk, wv, bv, wo, bo):
    out = np.empty((B, S, D), np.float32)
    for b in range(B):
        Q = (query[b] @ wq.T + bq).reshape(S, H, DK).transpose(1, 0, 2)
        K = (key[b] @ wk.T + bk).reshape(S, H, DK).transpose(1, 0, 2)
        Vv = (value[b] @ wv.T + bv).reshape(S, H, DK).transpose(1, 0, 2)
        sc = np.einsum("hqd,hkd->hqk", Q, K) / np.sqrt(np.float32(DK))
        sc = np.where(mask[b][None] == 0, -np.inf, sc)
        sc = sc - sc.max(axis=-1, keepdims=True)
        e = np.exp(sc)
        attn = e / e.sum(axis=-1, keepdims=True)
        ctx = np.einsum("hqk,hkd->hqd", attn, Vv)
        out[b] = ctx.transpose(1, 0, 2).reshape(S, D) @ wo.T + bo
    return out


def kernel(query, key, value, mask, wq, bq, wk, bk, wv, bv, wo, bo):
    global LAST_RESULT
    query = np.asarray(query, np.float32)
    key = np.asarray(key, np.float32)
    value = np.asarray(value, np.float32)
    mask = np.asarray(mask)
    wq, bq = np.asarray(wq, np.float32), np.asarray(bq, np.float32)
    wk, bk = np.asarray(wk, np.float32), np.asarray(bk, np.float32)
    wv, bv = np.asarray(wv, np.float32), np.asarray(bv, np.float32)
    wo, bo = np.asarray(wo, np.float32), np.asarray(bo, np.float32)

    tril = np.tril(np.ones((S, S), mask.dtype))
    if all((mask[b] == tril).all() for b in range(B)):
        causal = True
    elif (mask == 1).all():
        causal = False
    else:
        return _numpy_fallback(
            query, key, value, mask, wq, bq, wk, bk, wv, bv, wo, bo
        )

    from concourse.bass_utils import run_bass_kernel_spmd

    nc = get_program(causal)
    in_maps = _make_core_inputs(query, key, value, wq, bq, wk, bk, wv, bv, wo)
    trace = bool(int(os.environ.get("MHA_TRACE", "0")))
    res = run_bass_kernel_spmd(nc, in_maps, list(range(NCORES)), trace=trace)
    LAST_RESULT = res

    out = np.zeros((B, S, D), np.float32)
    for core in range(NCORES):
        b = core // G
        out[b] += np.asarray(res.results[core]["out"], np.float32)
    out += bo[None, None, :]
    return out



# revision 30
# speedup vs baseline: 1.0062x; 1.0062x over previous
"""MultiHeadAttention forward on 8 Trainium2 NeuronCores (Bass/Tile), v2.

Problem (hardcoded): B=2, S=2048, D=1024, H=16, DK=64, causal mask.

Sharding: data-parallel over batch (2) x tensor-parallel over heads
(4 heads per core).  core = 4*b + g handles batch b, heads [4g, 4g+4).
Host sums the 4 partial O outputs per batch and adds bo.

v2 restructure vs baseline:
  - Global software pipeline: next-chunk QKV-projection and prev-chunk
    O-projection matmuls are emitted as "filler" units interleaved into
    the attention j-loop, so the PE never starves while the Activation
    engine paces exp().
  - Causal handling: scores/exp/AV for diagonal k-tiles are
    range-restricted to valid queries; only the 128x128 diagonal block
    gets a tri-mask multiply (no DVE column zeroing).
  - Normalize chain: partition_broadcast reads the reciprocal row at
    partition 64 directly (no 1-partition staging DMA).
  - QT/KT in bf16; output partials in bf16 (host sums in f32).
  - DMA spread: wq/wk + k input on SP queue, q/v inputs + wv/wo + out
    on Pool queue, biases/tri/wo_lo on Act queue.
  - PSUM: sc 2x[128,1024] (4 banks) + ctx 2x[65,512] (2) + wk 2x[128,512]
    (2) = 8 banks.
  - Tail: last chunk's O-projection split into waves (pair-1 K=128, h0
    K=64 from cx, h1 K=64 from tmp via lane-shifted wo_lo) so no cx
    merge DMA sits on the critical path.
"""

import os
import sys

sys.path.insert(0, "/opt/trn_rl_repo")

import numpy as np

B, S, D, H = 2, 2048, 1024, 16
DK = D // H          # 64
NCORES = 8
G = 4                # heads per core
DG = G * DK          # 256 output dims per core
SC = 512             # seq chunk
NCH = S // SC        # 4
KI = D // 128        # 8 contraction chunks
NPAIR = 2            # head pairs per core
NST = SC // 128      # 4 seq 128-tiles per chunk

_programs = {}
LAST_RESULT = None


def _build_program(causal: bool, reps: int = 1):
    import concourse.bass as bass
    import concourse.tile as tile
    import concourse.mybir as mybir
    from concourse import bacc
    from contextlib import ExitStack
    from collections import deque

    F32 = mybir.dt.float32
    F32R = mybir.dt.float32r
    BF16 = mybir.dt.bfloat16
    AF = mybir.ActivationFunctionType

    nc = bacc.Bacc()
    # inputs are PRE-TILED host-side into their exact SBUF layouts so every
    # load is a flat contiguous DMA (128 descriptors x 4-8KB, no gather):
    #   qT/kT/vT: [128, NCH*8*SC]; chunk c occupies cols [c*8*SC,(c+1)*8*SC)
    #   with the ki-th 128-row d-block of that chunk at sub-cols
    #   [ki*SC,(ki+1)*SC) (h-major: ki = 4*h + a).
    qT_d = nc.dram_tensor("qT", [128, NCH * KI * SC], BF16, kind="ExternalInput")
    kT_d = nc.dram_tensor("kT", [128, NCH * KI * SC], BF16, kind="ExternalInput")
    vT_d = nc.dram_tensor("vT", [128, NCH * KI * SC], BF16, kind="ExternalInput")
    wq_d = nc.dram_tensor("wqT", [128, KI * DG], BF16, kind="ExternalInput")
    wk_d = nc.dram_tensor("wkT", [128, KI * DG], BF16, kind="ExternalInput")
    wv_d = nc.dram_tensor("wvT", [128, KI * DG], BF16, kind="ExternalInput")
    wo_d = nc.dram_tensor("woT", [128, 2 * D], BF16, kind="ExternalInput")
    bq_d = nc.dram_tensor("bq2", [128, NPAIR], F32, kind="ExternalInput")
    bk_d = nc.dram_tensor("bk2", [128, NPAIR], F32, kind="ExternalInput")
    bv_d = nc.dram_tensor("bvb", [128, DG], F32, kind="ExternalInput")
    tri_d = nc.dram_tensor("tri", [128, 128], BF16, kind="ExternalInput")
    out_d = nc.dram_tensor("out", [S, D], BF16, kind="ExternalOutput")

    with ExitStack() as ctx:
        tc = ctx.enter_context(tile.TileContext(nc))
        cpool = ctx.enter_context(tc.tile_pool(name="const", bufs=1))
        inpool = ctx.enter_context(tc.tile_pool(name="ins", bufs=5))
        qtpool = ctx.enter_context(tc.tile_pool(name="qt", bufs=1))
        ktpool = ctx.enter_context(tc.tile_pool(name="kt", bufs=1))
        vpool = ctx.enter_context(tc.tile_pool(name="v", bufs=1))
        cxpool = ctx.enter_context(tc.tile_pool(name="cx", bufs=1))
        prpool = ctx.enter_context(tc.tile_pool(name="probs", bufs=4))
        rcpool = ctx.enter_context(tc.tile_pool(name="recip", bufs=3))
        bcpool = ctx.enter_context(tc.tile_pool(name="bc", bufs=4))
        tmpool = ctx.enter_context(tc.tile_pool(name="tmp", bufs=3))
        outpool = ctx.enter_context(tc.tile_pool(name="osb", bufs=6))
        psS = ctx.enter_context(tc.tile_pool(name="psS", bufs=2, space="PSUM"))
        psC = ctx.enter_context(tc.tile_pool(name="psC", bufs=2, space="PSUM"))
        psW = ctx.enter_context(tc.tile_pool(name="psW", bufs=2, space="PSUM"))

        QT = {}
        KT = {}
        V = {}
        CX = {}
        INS = {}

        fillers = deque()   # hard: QK-proj of next chunk; flushed at boundary
        soft = deque()      # V-proj of next chunk + O-proj of prev chunk

        def drain(n=None, include_soft=True):
            k = (len(fillers) + (len(soft) if include_soft else 0)) if n is None else n
            for _ in range(k):
                if fillers:
                    fillers.popleft()()
                elif include_soft and soft:
                    soft.popleft()()
                else:
                    break

        # ---------- filler-unit builders (each closure = ~1 PE matmul) ----

        def qk_proj_fillers(w_sb, b_sb, in_sb, t, dst, c):
            """8 matmuls accumulating one [128,512] psum + bias-add -> bf16."""
            state = {}

            def mk(ki):
                def f():
                    if ki == 0:
                        state["ps"] = psW.tile(
                            [128, SC], F32, tag="wk", name="wkps"
                        )
                    nc.tensor.matmul(
                        state["ps"][:],
                        w_sb[:, (t * KI + ki) * 128 : (t * KI + ki + 1) * 128],
                        in_sb[:, ki * SC : (ki + 1) * SC],
                        start=(ki == 0),
                        stop=(ki == KI - 1),
                    )
                    if ki == KI - 1:
                        pool = qtpool if dst is QT else ktpool
                        tl = pool.tile(
                            [128, SC], BF16,
                            tag=f"{'q' if dst is QT else 'k'}{t}{c}",
                            name="qkt",
                        )
                        nc.vector.tensor_scalar_add(
                            tl[:], state["ps"][:], b_sb[:, t : t + 1]
                        )
                        dst[(t, c)] = tl

                return f

            return [mk(ki) for ki in range(KI)]

        def v_proj_fillers(wv_sb, bv_sb, in_sb, st, c):
            """8 matmuls -> [128,256] psum + assemble V[j] with ones cols."""
            j = c * NST + st
            state = {}

            def mk(ki):
                def f():
                    if ki == 0:
                        state["ps"] = psW.tile(
                            [128, SC], F32, tag="wk", name="wkps"
                        )
                    nc.tensor.matmul(
                        state["ps"][:, 0:DG],
                        in_sb[
                            :,
                            ki * SC + st * 128 : ki * SC + (st + 1) * 128,
                        ],
                        wv_sb[:, ki * DG : (ki + 1) * DG],
                        start=(ki == 0),
                        stop=(ki == KI - 1),
                    )
                    if ki == KI - 1:
                        vt = vpool.tile(
                            [128, G * 65], BF16, tag=f"v{j}", name="vt"
                        )
                        nc.vector.tensor_scalar(
                            vt[:].rearrange("p (h x) -> p h x", x=65)[:, :, 0:1],
                            bv_sb[:].rearrange("p (a b) -> p a b", b=1)[:, 0:G, :],
                            0.0,
                            1.0,
                            mybir.AluOpType.mult,
                            mybir.AluOpType.add,
                        )
                        nc.vector.tensor_add(
                            vt[:].rearrange("p (h x) -> p h x", x=65)[:, :, 1:65],
                            state["ps"][:, 0:DG].rearrange(
                                "p (h x) -> p h x", x=64
                            ),
                            bv_sb[:].rearrange("p (h x) -> p h x", x=64),
                        )
                        V[j] = vt

                return f

            return [mk(ki) for ki in range(KI)]

        def o_proj_fillers(c):
            """Partial O-projection for chunk c: per (st, half) one 2-matmul
            unit + psum->osb bf16 copy; DMA per st row."""
            units = []
            state = {}

            def mk(st, n):
                def f():
                    if n == 0:
                        state[st] = outpool.tile(
                            [128, D], BF16, tag="osb", name="osb"
                        )
                    ps = psW.tile([128, SC], F32, tag="wk", name="wkps")
                    for p in (1, 0):
                        nc.tensor.matmul(
                            ps[:],
                            CX[(p, c)][:, st * 128 : (st + 1) * 128],
                            wo_sb[:, p * D + n * SC : p * D + (n + 1) * SC],
                            start=(p == 1),
                            stop=(p == 0),
                        )
                    osb = state[st]
                    if n == 0:
                        nc.vector.tensor_copy(osb[:, 0:SC], ps[:])
                    else:
                        nc.vector.tensor_copy(osb[:, SC : 2 * SC], ps[:])
                        r0 = (c * NST + st) * 128
                        # sync queue, not gpsimd: keeps the gpsimd queue
                        # free for the latency-critical partition_broadcasts
                        nc.sync.dma_start(out_d[r0 : r0 + 128, :], osb[:])

                return f

            for st in range(NST):
                for n in range(2):
                    units.append(mk(st, n))
            return units

        def o_proj_tail(c):
            """Tail O-projection: pair-1 contractions first (its cx merges
            early), then pair-0 contractions + copies/DMAs spread over
            DVE/Act + SP/Pool/Act queues.  psum spread over psW/psC/psS."""
            pss = {}
            for n in range(2):
                pss[(0, n)] = psW.tile([128, SC], F32, tag="wk", name="otps")
                pss[(1, n)] = psC.tile([128, SC], F32, tag="ctx", name="otps")
            for st in (2, 3):
                big = psS.tile([128, 2 * SC], F32, tag="sc", name="otps")
                pss[(st, 0)] = big[:, 0:SC]
                pss[(st, 1)] = big[:, SC : 2 * SC]
            # wave A: pair-1 contractions for sts not on psC (ready early)
            for st in (0, 2, 3):
                for n in range(2):
                    nc.tensor.matmul(
                        pss[(st, n)],
                        CX[(1, c)][:, st * 128 : (st + 1) * 128],
                        wo_sb[:, D + n * SC : D + (n + 1) * SC],
                        start=True,
                        stop=False,
                        skip_group_check=True,
                    )
            # st1 on psC slots (free once the normalize consumed ctx0/ctx1)
            for n in range(2):
                nc.tensor.matmul(
                    pss[(1, n)],
                    CX[(1, c)][:, 128:256],
                    wo_sb[:, D + n * SC : D + (n + 1) * SC],
                    start=True,
                    stop=False,
                    skip_group_check=True,
                )
            # wave B: pair-0 contractions on merged cx + copies (DVE/Act
            # alternating) + bf16 out DMAs on sync/gpsimd
            ei = 0
            copies = (nc.vector, nc.scalar, nc.vector, nc.scalar,
                      nc.vector, nc.scalar, nc.vector, nc.scalar)
            # sync+scalar only (both HW-DGE): keeping the tail off the
            # gpsimd SW-DGE queue makes its end-of-kernel DRAIN instant
            dmas = (nc.sync, nc.scalar, nc.sync, nc.scalar,
                    nc.sync, nc.scalar, nc.sync, nc.scalar)
            osbs = {}
            for st in (0, 2, 3, 1):
                osbs[st] = outpool.tile([128, D], BF16, tag="osb", name="osb")
            for st in (0, 2, 3, 1):
                osb = osbs[st]
                r0 = (c * NST + st) * 128
                for n in range(2):
                    nc.tensor.matmul(
                        pss[(st, n)],
                        CX[(0, c)][:, st * 128 : (st + 1) * 128],
                        wo_sb[:, n * SC : (n + 1) * SC],
                        start=False,
                        stop=True,
                        skip_group_check=True,
                    )
                    ceng = copies[ei]
                    if ceng is nc.scalar:
                        ceng.copy(osb[:, n * SC : (n + 1) * SC], pss[(st, n)])
                    else:
                        ceng.tensor_copy(
                            osb[:, n * SC : (n + 1) * SC], pss[(st, n)]
                        )
                    dmas[ei].dma_start(
                        out_d[r0 : r0 + 128, n * SC : (n + 1) * SC],
                        osb[:, n * SC : (n + 1) * SC],
                    )
                    ei += 1

        def stream_in(c, tag, t_d, eng):
            t_sb = inpool.tile(
                [128, KI * SC], BF16, tag=tag, name="t_sb", bufs=3
            )
            eng.dma_start(
                t_sb[:], t_d[:, c * KI * SC : (c + 1) * KI * SC]
            )
            return t_sb

        def emit_input_dmas(c):
            # all on sync: a gpsimd dma_start unloads the
            # partition_broadcast library (MODIFY_POOL_CONFIG) and the
            # reload waits on the gpsimd DMA queue drain — never mix them
            qin = stream_in(c, "qin", qT_d, nc.sync)
            kin = stream_in(c, "kin", kT_d, nc.sync)
            vin = stream_in(c, "vin", vT_d, nc.sync)
            return qin, kin, vin

        # ---------- startup -------------------------------------------------

        if reps > 1:
            ctx.enter_context(
                tc.For_i(
                    0,
                    reps,
                    1,
                    hint_engines=(
                        mybir.EngineType.PE,
                        mybir.EngineType.Activation,
                        mybir.EngineType.DVE,
                        mybir.EngineType.SP,
                        mybir.EngineType.Pool,
                    ),
                )
            )

        wq_sb = cpool.tile([128, KI * DG], BF16, tag="wq")
        wk_sb = cpool.tile([128, KI * DG], BF16, tag="wk")
        wv_sb = cpool.tile([128, KI * DG], BF16, tag="wv")
        wo_sb = cpool.tile([128, 2 * D], BF16, tag="wo")
        # wq/wk are t-major host-side; pair t=1 is projected first, so its
        # half loads first
        for lo, hi in ((1024, 1536), (1536, 2048), (0, 1024)):
            nc.sync.dma_start(wq_sb[:, lo:hi], wq_d[:, lo:hi])
        bq_sb = cpool.tile([128, NPAIR], F32, tag="bq")
        bk_sb = cpool.tile([128, NPAIR], F32, tag="bk")
        bv_sb = cpool.tile([128, DG], F32, tag="bv")
        tri_sb = cpool.tile([128, 128], BF16, tag="tri")
        nc.scalar.dma_start(bq_sb[:], bq_d[:])
        nc.scalar.dma_start(bk_sb[:], bk_d[:])
        nc.scalar.dma_start(bv_sb[:], bv_d[:])
        nc.scalar.dma_start(tri_sb[:], tri_d[:])

        # chunk-0 inputs, split into pieces so the first projection fillers
        # unblock as soon as their ki-blocks land
        qin = inpool.tile([128, KI * SC], BF16, tag="qin", name="qin", bufs=3)
        for lo, hi in ((0, 1), (1, 2), (2, 4), (4, 6), (6, 8)):
            nc.gpsimd.dma_start(
                qin[:, lo * SC : hi * SC], qT_d[:, lo * SC : hi * SC]
            )
        nc.sync.dma_start(wk_sb[:, 1024:2048], wk_d[:, 1024:2048])
        kin = inpool.tile([128, KI * SC], BF16, tag="kin", name="kin", bufs=3)
        for (lo, hi), eng in (((0, 4), nc.sync), ((4, 8), nc.gpsimd)):
            eng.dma_start(
                kin[:, lo * SC : hi * SC], kT_d[:, lo * SC : hi * SC]
            )
        nc.sync.dma_start(wk_sb[:, 0:1024], wk_d[:, 0:1024])
        nc.gpsimd.dma_start(wv_sb[:], wv_d[:])
        # vin/wo ride the Act hardware-DGE queue: it is idle before the
        # first exps, giving a third parallel transfer queue at startup
        vin = stream_in(0, "vin", vT_d, nc.scalar)
        nc.scalar.dma_start(wo_sb[:], wo_d[:])

        # chunk-0: pair-1 Q/K projections drain now (attention needs them);
        # V + pair-0 projections become in-loop fillers.  V goes FIRST in
        # the hard queue: pair-1's AVs need V[0..3] right away, and they
        # must not sit behind chunk-1 projections that are gated on
        # chunk-1 input DMAs.
        fillers.extend(qk_proj_fillers(wq_sb, bq_sb, qin, 1, QT, 0))
        fillers.extend(qk_proj_fillers(wk_sb, bk_sb, kin, 1, KT, 0))
        drain(include_soft=False)
        for st in range(NST):
            fillers.extend(v_proj_fillers(wv_sb, bv_sb, vin, st, 0))
        fillers.extend(qk_proj_fillers(wq_sb, bq_sb, qin, 0, QT, 0))
        fillers.extend(qk_proj_fillers(wk_sb, bk_sb, kin, 0, KT, 0))
        if not causal:
            # full-width attention reads every chunk's K/V from chunk 0 on:
            # project everything up front (correctness over overlap)
            drain()
            for cc in range(1, NCH):
                qin, kin, vin = emit_input_dmas(cc)
                for t in (1, 0):
                    fillers.extend(
                        qk_proj_fillers(wq_sb, bq_sb, qin, t, QT, cc)
                    )
                    fillers.extend(
                        qk_proj_fillers(wk_sb, bk_sb, kin, t, KT, cc)
                    )
                for st in range(NST):
                    fillers.extend(v_proj_fillers(wv_sb, bv_sb, vin, st, cc))
                drain()

        # ---------- main loop over chunks ---------------------------------

        for c in range(NCH):
            # hard queue: QK-proj of c+1 (first-processed pair first);
            # soft queue: V-proj of c+1, then O-proj of c-1.
            # inputs are prefetched TWO chunks ahead (ring bufs=3) so the
            # c+1 projection fillers never stall the attention stream on
            # input arrival
            if causal and c == 0:
                INS[1] = emit_input_dmas(1)
                if NCH > 2:
                    INS[2] = emit_input_dmas(2)
            elif causal and c + 2 < NCH:
                INS[c + 2] = emit_input_dmas(c + 2)
            if causal and c + 1 < NCH:
                qin, kin, vin = INS[c + 1]
                for t in (1, 0):
                    fillers.extend(
                        qk_proj_fillers(wq_sb, bq_sb, qin, t, QT, c + 1)
                    )
                    fillers.extend(
                        qk_proj_fillers(wk_sb, bk_sb, kin, t, KT, c + 1)
                    )
                for st in range(NST):
                    soft.extend(v_proj_fillers(wv_sb, bv_sb, vin, st, c + 1))
            if c > 0:
                soft.extend(o_proj_fillers(c - 1))

            # attention j-loop for chunk c, per head pair
            for p in (1, 0):
                while ((p, c) not in QT or (p, c) not in KT) and (
                    fillers or soft
                ):
                    drain(1)
                njt = NST * (c + 1) if causal else NST * NCH
                ctx0 = psC.tile([65, SC], F32, tag="ctx")
                ctx1 = psC.tile([65, SC], F32, tag="ctx")
                h0, h1 = 2 * p, 2 * p + 1
                pending = None

                def emit_av(j, probs, first, last, q0):
                    nc.tensor.matmul(
                        ctx0[:, q0:SC],
                        V[j][:, 65 * h0 : 65 * h0 + 65],
                        probs[:, q0:SC],
                        start=first,
                        stop=last,
                        skip_group_check=True,
                    )
                    nc.tensor.matmul(
                        ctx1[:, q0:SC],
                        V[j][:, 65 * h1 : 65 * h1 + 65],
                        probs[:, SC + q0 : 2 * SC],
                        start=first,
                        stop=last,
                        skip_group_check=True,
                    )

                for j in range(njt):
                    jc, jt = divmod(j, NST)
                    diag = causal and jc == c
                    m = jt if diag else 0
                    q0 = 128 * m  # first valid query col in this chunk
                    scp = psS.tile([128, 2 * SC], F32, tag="sc")
                    nc.tensor.matmul(
                        scp[:, q0:SC],
                        KT[(p, jc)][0:64, jt * 128 : (jt + 1) * 128],
                        QT[(p, c)][0:64, q0:SC],
                        start=True,
                        stop=True,
                        tile_position=(0, 0),
                    )
                    nc.tensor.matmul(
                        scp[:, SC + q0 : 2 * SC],
                        KT[(p, jc)][64:128, jt * 128 : (jt + 1) * 128],
                        QT[(p, c)][64:128, q0:SC],
                        start=True,
                        stop=True,
                        tile_position=(64, 0),
                    )
                    probs = prpool.tile([128, 2 * SC], BF16, tag="probs")
                    if m == 0:
                        nc.scalar.activation(
                            probs[:], scp[:], AF.Exp, scale=0.125
                        )
                    else:
                        nc.scalar.activation(
                            probs[:, q0:SC], scp[:, q0:SC], AF.Exp, scale=0.125
                        )
                        nc.scalar.activation(
                            probs[:, SC + q0 : 2 * SC],
                            scp[:, SC + q0 : 2 * SC],
                            AF.Exp,
                            scale=0.125,
                        )
                    if diag:
                        # tri-mask the 128x128 diagonal block of both heads
                        for off in (0, SC):
                            lo = off + q0
                            nc.vector.tensor_mul(
                                probs[:, lo : lo + 128],
                                probs[:, lo : lo + 128],
                                tri_sb[:],
                            )
                    if pending is not None:
                        while pending[0] not in V and (fillers or soft):
                            drain(1)
                        emit_av(*pending)
                        drain(1)
                    pending = (j, probs, j == 0, j == njt - 1, q0)
                while pending[0] not in V and (fillers or soft):
                    drain(1)
                jL, probsL, firstL, lastL, q0L = pending
                nc.tensor.matmul(
                    ctx0[:, q0L:SC],
                    V[jL][:, 65 * h0 : 65 * h0 + 65],
                    probsL[:, q0L:SC],
                    start=firstL,
                    stop=lastL,
                    skip_group_check=True,
                )
                # ---- eager normalize v3: copy ctx psum -> SBUF raw f32
                # immediately (the copy is the ONLY psum reader, so psC
                # frees ~0.6us after the last AV instead of after the
                # whole recip->broadcast->mul chain, unblocking the next
                # pair's AVs).  Reciprocals use the ~5x faster approx
                # custom-DVE op.  The very last pair skips the copies and
                # normalizes straight from psum: nothing is waiting on the
                # psC banks there, and the tail wants the shortest chain.
                last_pair = p == 0 and c == NCH - 1
                rec = rcpool.tile([1, 2 * SC], F32, tag="recip")
                if last_pair:
                    raw0 = ctx0
                else:
                    raw0 = tmpool.tile([65, SC], F32, tag="raw0")
                    nc.vector.tensor_copy(raw0[:], ctx0[0:65, :])
                nc.vector.reciprocal_approx_fast(
                    rec[0:1, 0:SC], raw0[0:1, :]
                )
                nc.tensor.matmul(
                    ctx1[:, q0L:SC],
                    V[jL][:, 65 * h1 : 65 * h1 + 65],
                    probsL[:, SC + q0L : 2 * SC],
                    start=firstL,
                    stop=lastL,
                    skip_group_check=True,
                )
                if last_pair:
                    raw1 = ctx1
                else:
                    # on DVE, not Act: the Act queue paces the j-loop exps
                    # and a copy there delays the next pair's first exp
                    raw1 = tmpool.tile([65, SC], F32, tag="raw1")
                    nc.vector.tensor_copy(raw1[:], ctx1[0:65, :])
                nc.vector.reciprocal_approx_fast(
                    rec[0:1, SC : 2 * SC], raw1[0:1, :]
                )
                if p == 0:
                    # flush next-chunk projections now so their DVE
                    # bias-adds queue ahead of this pair's normalize muls
                    drain(include_soft=False)

                cx = cxpool.tile([128, SC], BF16, tag=f"cx{p}{c}")
                bc0 = bcpool.tile([65, SC], F32, tag="bc")
                bc1 = bcpool.tile([65, SC], F32, tag="bc")
                cxa = tmpool.tile([65, SC], BF16, tag="cxa")
                tmp = tmpool.tile([65, SC], BF16, tag="tmp")
                if last_pair:
                    # half-split pipeline: the tail's wave-B q-tiles 0/1
                    # only need cx cols [0,256), so their stationaries
                    # unblock a chain-half earlier
                    for lo, hi in ((0, SC // 2), (SC // 2, SC)):
                        nc.gpsimd.partition_broadcast(
                            bc0[:, lo:hi], rec[0:1, lo:hi]
                        )
                        nc.gpsimd.partition_broadcast(
                            bc1[:, lo:hi], rec[0:1, SC + lo : SC + hi]
                        )
                        nc.vector.tensor_mul(
                            cxa[0:65, lo:hi], raw0[0:65, lo:hi],
                            bc0[0:65, lo:hi],
                        )
                        nc.vector.tensor_mul(
                            tmp[0:65, lo:hi], raw1[0:65, lo:hi],
                            bc1[0:65, lo:hi],
                        )
                        nc.sync.dma_start(cx[0:64, lo:hi], cxa[1:65, lo:hi])
                        nc.scalar.dma_start(
                            cx[64:128, lo:hi], tmp[1:65, lo:hi]
                        )
                else:
                    nc.gpsimd.partition_broadcast(bc0[:], rec[0:1, 0:SC])
                    nc.gpsimd.partition_broadcast(
                        bc1[:], rec[0:1, SC : 2 * SC]
                    )
                    nc.vector.tensor_mul(
                        cxa[0:65, :], raw0[0:65, :], bc0[0:65, :]
                    )
                    nc.vector.tensor_mul(
                        tmp[0:65, :], raw1[0:65, :], bc1[0:65, :]
                    )
                    nc.sync.dma_start(cx[0:64, :], cxa[1:65, :])
                    nc.sync.dma_start(cx[64:128, :], tmp[1:65, :])
                CX[(p, c)] = cx

            # boundary: flush hard queue (QK proj of c+1); soft carries over
            drain(include_soft=False)

        drain()
        o_proj_tail(NCH - 1)

    nc.finalize()
    return nc


def get_program(causal: bool):
    if causal not in _programs:
        _programs[causal] = _build_program(causal)
    return _programs[causal]


def _tile_seq(xT):
    """[D, S] -> [128, NCH*KI*SC]: chunk-major, then 128-row d-block (ki)."""
    return np.ascontiguousarray(
        xT.reshape(KI, 128, NCH, SC).transpose(1, 2, 0, 3).reshape(128, -1)
    )


def _tile_w(wT, nblk):
    """[nblk*128, M] -> [128, nblk*M]: 128-row block i at cols [i*M,(i+1)*M)."""
    m = wT.shape[1]
    return np.ascontiguousarray(
        wT.reshape(nblk, 128, m).transpose(1, 0, 2).reshape(128, nblk * m)
    )


def _tile_w_tmaj(wT):
    """[D, DG] -> [128, 2*KI*128], t-major: (t, ki) block at
    cols [(t*KI+ki)*128, ...)."""
    return np.ascontiguousarray(
        wT.reshape(KI, 128, NPAIR, 128)
        .transpose(1, 2, 0, 3)
        .reshape(128, NPAIR * KI * 128)
    )


def _make_core_inputs(query, key, value, wq, bq, wk, bk, wv, bv, wo):
    import ml_dtypes

    bf16 = ml_dtypes.bfloat16
    f32 = np.float32
    tri = np.triu(np.ones((128, 128), f32)).astype(bf16)
    in_maps = []
    qTt = [_tile_seq(query[b].T.astype(bf16)) for b in range(B)]
    kTt = [_tile_seq(key[b].T.astype(bf16)) for b in range(B)]
    vTt = [_tile_seq(value[b].T.astype(bf16)) for b in range(B)]
    for core in range(NCORES):
        b, g = divmod(core, G)
        sl = slice(g * DG, (g + 1) * DG)
        in_maps.append(
            {
                "qT": qTt[b],
                "kT": kTt[b],
                "vT": vTt[b],
                "wqT": _tile_w_tmaj(wq[sl, :].T.astype(bf16)),
                "wkT": _tile_w_tmaj(wk[sl, :].T.astype(bf16)),
                "wvT": _tile_w(wv[sl, :].T.astype(bf16), KI),
                "woT": _tile_w(wo[:, sl].T.astype(bf16), 2),
                "bq2": np.ascontiguousarray(bq[sl].reshape(NPAIR, 128).T, f32),
                "bk2": np.ascontiguousarray(bk[sl].reshape(NPAIR, 128).T, f32),
                "bvb": np.ascontiguousarray(
                    np.broadcast_to(bv[sl], (128, DG)), f32
                ),
                "tri": tri,
            }
        )
    return in_maps


def _numpy_fallback(query, key, value, mask, wq, bq, wk, bk, wv, bv, wo, bo):
    out = np.empty((B, S, D), np.float32)
    for b in range(B):
        Q = (query[b] @ wq.T + bq).reshape(S, H, DK).transpose(1, 0, 2)
        K = (key[b] @ wk.T + bk).reshape(S, H, DK).transpose(1, 0, 2)
        Vv = (value[b] @ wv.T + bv).reshape(S, H, DK).transpose(1, 0, 2)
        sc = np.einsum("hqd,hkd->hqk", Q, K) / np.sqrt(np.float32(DK))
        sc = np.where(mask[b][None] == 0, -np.inf, sc)
        sc = sc - sc.max(axis=-1, keepdims=True)
        e = np.exp(sc)
        attn = e / e.sum(axis=-1, keepdims=True)
        ctx = np.einsum("hqk,hkd->hqd", attn, Vv)
        out[b] = ctx.transpose(1, 0, 2).reshape(S, D) @ wo.T + bo
    return out


def kernel(query, key, value, mask, wq, bq, wk, bk, wv, bv, wo, bo):
    global LAST_RESULT
    query = np.asarray(query, np.float32)
    key = np.asarray(key, np.float32)
    value = np.asarray(value, np.float32)
    mask = np.asarray(mask)
    wq, bq = np.asarray(wq, np.float32), np.asarray(bq, np.float32)
    wk, bk = np.asarray(wk, np.float32), np.asarray(bk, np.float32)
    wv, bv = np.asarray(wv, np.float32), np.asarray(bv, np.float32)
    wo, bo = np.asarray(wo, np.float32), np.asarray(bo, np.float32)

    tril = np.tril(np.ones((S, S), mask.dtype))
    if all((mask[b] == tril).all() for b in range(B)):
        causal = True
    elif (mask == 1).all():
        causal = False
    else:
        return _numpy_fallback(
            query, key, value, mask, wq, bq, wk, bk, wv, bv, wo, bo
        )

    from concourse.bass_utils import run_bass_kernel_spmd

    nc = get_program(causal)
    in_maps = _make_core_inputs(query, key, value, wq, bq, wk, bk, wv, bv, wo)
    trace = bool(int(os.environ.get("MHA_TRACE", "0")))
    res = run_bass_kernel_spmd(nc, in_maps, list(range(NCORES)), trace=trace)
    LAST_RESULT = res

    out = np.zeros((B, S, D), np.float32)
    for core in range(NCORES):
        b = core // G
        out[b] += np.asarray(res.results[core]["out"], np.float32)
    out += bo[None, None, :]
    return out



# revision 32
# speedup vs baseline: 1.0175x; 1.0113x over previous
"""MultiHeadAttention forward on 8 Trainium2 NeuronCores (Bass/Tile), v2.

Problem (hardcoded): B=2, S=2048, D=1024, H=16, DK=64, causal mask.

Sharding: data-parallel over batch (2) x tensor-parallel over heads
(4 heads per core).  core = 4*b + g handles batch b, heads [4g, 4g+4).
Host sums the 4 partial O outputs per batch and adds bo.

v2 restructure vs baseline:
  - Global software pipeline: next-chunk QKV-projection and prev-chunk
    O-projection matmuls are emitted as "filler" units interleaved into
    the attention j-loop, so the PE never starves while the Activation
    engine paces exp().
  - Causal handling: scores/exp/AV for diagonal k-tiles are
    range-restricted to valid queries; only the 128x128 diagonal block
    gets a tri-mask multiply (no DVE column zeroing).
  - Normalize chain: partition_broadcast reads the reciprocal row at
    partition 64 directly (no 1-partition staging DMA).
  - QT/KT in bf16; output partials in bf16 (host sums in f32).
  - DMA spread: wq/wk + k input on SP queue, q/v inputs + wv/wo + out
    on Pool queue, biases/tri/wo_lo on Act queue.
  - PSUM: sc 2x[128,1024] (4 banks) + ctx 2x[65,512] (2) + wk 2x[128,512]
    (2) = 8 banks.
  - Tail: last chunk's O-projection split into waves (pair-1 K=128, h0
    K=64 from cx, h1 K=64 from tmp via lane-shifted wo_lo) so no cx
    merge DMA sits on the critical path.
"""

import os
import sys

sys.path.insert(0, "/opt/trn_rl_repo")

import numpy as np

B, S, D, H = 2, 2048, 1024, 16
DK = D // H          # 64
NCORES = 8
G = 4                # heads per core
DG = G * DK          # 256 output dims per core
SC = 512             # seq chunk
NCH = S // SC        # 4
KI = D // 128        # 8 contraction chunks
NPAIR = 2            # head pairs per core
NST = SC // 128      # 4 seq 128-tiles per chunk

_programs = {}
LAST_RESULT = None


def _build_program(causal: bool, reps: int = 1):
    import concourse.bass as bass
    import concourse.tile as tile
    import concourse.mybir as mybir
    from concourse import bacc
    from contextlib import ExitStack
    from collections import deque

    F32 = mybir.dt.float32
    F32R = mybir.dt.float32r
    BF16 = mybir.dt.bfloat16
    AF = mybir.ActivationFunctionType

    nc = bacc.Bacc()
    # inputs are PRE-TILED host-side into their exact SBUF layouts so every
    # load is a flat contiguous DMA (128 descriptors x 4-8KB, no gather):
    #   qT/kT/vT: [128, NCH*8*SC]; chunk c occupies cols [c*8*SC,(c+1)*8*SC)
    #   with the ki-th 128-row d-block of that chunk at sub-cols
    #   [ki*SC,(ki+1)*SC) (h-major: ki = 4*h + a).
    qT_d = nc.dram_tensor("qT", [128, NCH * KI * SC], BF16, kind="ExternalInput")
    kT_d = nc.dram_tensor("kT", [128, NCH * KI * SC], BF16, kind="ExternalInput")
    vT_d = nc.dram_tensor("vT", [128, NCH * KI * SC], BF16, kind="ExternalInput")
    wq_d = nc.dram_tensor("wqT", [128, KI * DG], BF16, kind="ExternalInput")
    wk_d = nc.dram_tensor("wkT", [128, KI * DG], BF16, kind="ExternalInput")
    wv_d = nc.dram_tensor("wvT", [128, KI * DG], BF16, kind="ExternalInput")
    wo_d = nc.dram_tensor("woT", [128, 2 * D], BF16, kind="ExternalInput")
    bq_d = nc.dram_tensor("bq2", [128, NPAIR], F32, kind="ExternalInput")
    bk_d = nc.dram_tensor("bk2", [128, NPAIR], F32, kind="ExternalInput")
    bv_d = nc.dram_tensor("bvb", [128, DG], F32, kind="ExternalInput")
    tri_d = nc.dram_tensor("tri", [128, 128], BF16, kind="ExternalInput")
    out_d = nc.dram_tensor("out", [S, D], BF16, kind="ExternalOutput")

    with ExitStack() as ctx:
        tc = ctx.enter_context(tile.TileContext(nc))
        cpool = ctx.enter_context(tc.tile_pool(name="const", bufs=1))
        inpool = ctx.enter_context(tc.tile_pool(name="ins", bufs=5))
        qtpool = ctx.enter_context(tc.tile_pool(name="qt", bufs=1))
        ktpool = ctx.enter_context(tc.tile_pool(name="kt", bufs=1))
        vpool = ctx.enter_context(tc.tile_pool(name="v", bufs=1))
        cxpool = ctx.enter_context(tc.tile_pool(name="cx", bufs=1))
        prpool = ctx.enter_context(tc.tile_pool(name="probs", bufs=4))
        rcpool = ctx.enter_context(tc.tile_pool(name="recip", bufs=3))
        bcpool = ctx.enter_context(tc.tile_pool(name="bc", bufs=4))
        tmpool = ctx.enter_context(tc.tile_pool(name="tmp", bufs=3))
        outpool = ctx.enter_context(tc.tile_pool(name="osb", bufs=6))
        psS = ctx.enter_context(tc.tile_pool(name="psS", bufs=2, space="PSUM"))
        psC = ctx.enter_context(tc.tile_pool(name="psC", bufs=2, space="PSUM"))
        psW = ctx.enter_context(tc.tile_pool(name="psW", bufs=2, space="PSUM"))

        QT = {}
        KT = {}
        V = {}
        CX = {}
        INS = {}

        fillers = deque()   # hard: QK-proj of next chunk; flushed at boundary
        soft = deque()      # V-proj of next chunk + O-proj of prev chunk

        def drain(n=None, include_soft=True):
            k = (len(fillers) + (len(soft) if include_soft else 0)) if n is None else n
            for _ in range(k):
                if fillers:
                    fillers.popleft()()
                elif include_soft and soft:
                    soft.popleft()()
                else:
                    break

        # ---------- filler-unit builders (each closure = ~1 PE matmul) ----

        def qk_proj_fillers(w_sb, b_sb, in_sb, t, dst, c):
            """8 matmuls accumulating one [128,512] psum + bias-add -> bf16."""
            state = {}

            def mk(ki):
                def f():
                    if ki == 0:
                        state["ps"] = psW.tile(
                            [128, SC], F32, tag="wk", name="wkps"
                        )
                    nc.tensor.matmul(
                        state["ps"][:],
                        w_sb[:, (t * KI + ki) * 128 : (t * KI + ki + 1) * 128],
                        in_sb[:, ki * SC : (ki + 1) * SC],
                        start=(ki == 0),
                        stop=(ki == KI - 1),
                    )
                    if ki == KI - 1:
                        pool = qtpool if dst is QT else ktpool
                        tl = pool.tile(
                            [128, SC], BF16,
                            tag=f"{'q' if dst is QT else 'k'}{t}{c}",
                            name="qkt",
                        )
                        nc.vector.tensor_scalar_add(
                            tl[:], state["ps"][:], b_sb[:, t : t + 1]
                        )
                        dst[(t, c)] = tl

                return f

            return [mk(ki) for ki in range(KI)]

        def v_proj_fillers(wv_sb, bv_sb, in_sb, st, c):
            """8 matmuls -> [128,256] psum + assemble V[j] with ones cols."""
            j = c * NST + st
            state = {}

            def mk(ki):
                def f():
                    if ki == 0:
                        state["ps"] = psW.tile(
                            [128, SC], F32, tag="wk", name="wkps"
                        )
                    nc.tensor.matmul(
                        state["ps"][:, 0:DG],
                        in_sb[
                            :,
                            ki * SC + st * 128 : ki * SC + (st + 1) * 128,
                        ],
                        wv_sb[:, ki * DG : (ki + 1) * DG],
                        start=(ki == 0),
                        stop=(ki == KI - 1),
                    )
                    if ki == KI - 1:
                        vt = vpool.tile(
                            [128, G * 65], BF16, tag=f"v{j}", name="vt"
                        )
                        nc.vector.tensor_scalar(
                            vt[:].rearrange("p (h x) -> p h x", x=65)[:, :, 0:1],
                            bv_sb[:].rearrange("p (a b) -> p a b", b=1)[:, 0:G, :],
                            0.0,
                            1.0,
                            mybir.AluOpType.mult,
                            mybir.AluOpType.add,
                        )
                        nc.vector.tensor_add(
                            vt[:].rearrange("p (h x) -> p h x", x=65)[:, :, 1:65],
                            state["ps"][:, 0:DG].rearrange(
                                "p (h x) -> p h x", x=64
                            ),
                            bv_sb[:].rearrange("p (h x) -> p h x", x=64),
                        )
                        V[j] = vt

                return f

            return [mk(ki) for ki in range(KI)]

        def o_proj_fillers(c):
            """Partial O-projection for chunk c: per (st, half) one 2-matmul
            unit + psum->osb bf16 copy; DMA per st row."""
            units = []
            state = {}

            def mk(st, n):
                def f():
                    if n == 0:
                        state[st] = outpool.tile(
                            [128, D], BF16, tag="osb", name="osb"
                        )
                    ps = psW.tile([128, SC], F32, tag="wk", name="wkps")
                    for p in (1, 0):
                        nc.tensor.matmul(
                            ps[:],
                            CX[(p, c)][:, st * 128 : (st + 1) * 128],
                            wo_sb[:, p * D + n * SC : p * D + (n + 1) * SC],
                            start=(p == 1),
                            stop=(p == 0),
                        )
                    osb = state[st]
                    if n == 0:
                        nc.vector.tensor_copy(osb[:, 0:SC], ps[:])
                    else:
                        nc.vector.tensor_copy(osb[:, SC : 2 * SC], ps[:])
                        r0 = (c * NST + st) * 128
                        # sync queue, not gpsimd: keeps the gpsimd queue
                        # free for the latency-critical partition_broadcasts
                        nc.sync.dma_start(out_d[r0 : r0 + 128, :], osb[:])

                return f

            for st in range(NST):
                for n in range(2):
                    units.append(mk(st, n))
            return units

        def o_proj_tail(c):
            """Tail O-projection: pair-1 contractions first (its cx merges
            early), then pair-0 contractions + copies/DMAs spread over
            DVE/Act + SP/Pool/Act queues.  psum spread over psW/psC/psS."""
            pss = {}
            for n in range(2):
                pss[(0, n)] = psW.tile([128, SC], F32, tag="wk", name="otps")
                pss[(1, n)] = psC.tile([128, SC], F32, tag="ctx", name="otps")
            for st in (2, 3):
                big = psS.tile([128, 2 * SC], F32, tag="sc", name="otps")
                pss[(st, 0)] = big[:, 0:SC]
                pss[(st, 1)] = big[:, SC : 2 * SC]
            # wave A: pair-1 contractions for sts not on psC (ready early)
            for st in (0, 2, 3):
                for n in range(2):
                    nc.tensor.matmul(
                        pss[(st, n)],
                        CX[(1, c)][:, st * 128 : (st + 1) * 128],
                        wo_sb[:, D + n * SC : D + (n + 1) * SC],
                        start=True,
                        stop=False,
                        skip_group_check=True,
                    )
            # st1 on psC slots (free once the normalize consumed ctx0/ctx1)
            for n in range(2):
                nc.tensor.matmul(
                    pss[(1, n)],
                    CX[(1, c)][:, 128:256],
                    wo_sb[:, D + n * SC : D + (n + 1) * SC],
                    start=True,
                    stop=False,
                    skip_group_check=True,
                )
            # wave B: pair-0 contractions on merged cx + copies (DVE/Act
            # alternating) + bf16 out DMAs on sync/gpsimd
            ei = 0
            copies = (nc.vector, nc.scalar, nc.vector, nc.scalar,
                      nc.vector, nc.scalar, nc.vector, nc.scalar)
            dmas = (nc.sync, nc.gpsimd, nc.sync, nc.gpsimd,
                    nc.sync, nc.gpsimd, nc.sync, nc.gpsimd)
            osbs = {}
            for st in (0, 2, 3, 1):
                osbs[st] = outpool.tile([128, D], BF16, tag="osb", name="osb")
            for st in (0, 2, 3, 1):
                osb = osbs[st]
                r0 = (c * NST + st) * 128
                for n in range(2):
                    nc.tensor.matmul(
                        pss[(st, n)],
                        CX[(0, c)][:, st * 128 : (st + 1) * 128],
                        wo_sb[:, n * SC : (n + 1) * SC],
                        start=False,
                        stop=True,
                        skip_group_check=True,
                    )
                    ceng = copies[ei]
                    if ceng is nc.scalar:
                        ceng.copy(osb[:, n * SC : (n + 1) * SC], pss[(st, n)])
                    else:
                        ceng.tensor_copy(
                            osb[:, n * SC : (n + 1) * SC], pss[(st, n)]
                        )
                    dmas[ei].dma_start(
                        out_d[r0 : r0 + 128, n * SC : (n + 1) * SC],
                        osb[:, n * SC : (n + 1) * SC],
                    )
                    ei += 1

        def stream_in(c, tag, t_d, eng):
            t_sb = inpool.tile(
                [128, KI * SC], BF16, tag=tag, name="t_sb", bufs=3
            )
            eng.dma_start(
                t_sb[:], t_d[:, c * KI * SC : (c + 1) * KI * SC]
            )
            return t_sb

        def emit_input_dmas(c):
            # all on sync: a gpsimd dma_start unloads the
            # partition_broadcast library (MODIFY_POOL_CONFIG) and the
            # reload waits on the gpsimd DMA queue drain — never mix them
            qin = stream_in(c, "qin", qT_d, nc.sync)
            kin = stream_in(c, "kin", kT_d, nc.sync)
            vin = stream_in(c, "vin", vT_d, nc.sync)
            return qin, kin, vin

        # ---------- startup -------------------------------------------------

        if reps > 1:
            ctx.enter_context(
                tc.For_i(
                    0,
                    reps,
                    1,
                    hint_engines=(
                        mybir.EngineType.PE,
                        mybir.EngineType.Activation,
                        mybir.EngineType.DVE,
                        mybir.EngineType.SP,
                        mybir.EngineType.Pool,
                    ),
                )
            )

        wq_sb = cpool.tile([128, KI * DG], BF16, tag="wq")
        wk_sb = cpool.tile([128, KI * DG], BF16, tag="wk")
        wv_sb = cpool.tile([128, KI * DG], BF16, tag="wv")
        wo_sb = cpool.tile([128, 2 * D], BF16, tag="wo")
        # wq/wk are t-major host-side; pair t=1 is projected first, so its
        # half loads first
        for lo, hi in ((1024, 1536), (1536, 2048), (0, 1024)):
            nc.sync.dma_start(wq_sb[:, lo:hi], wq_d[:, lo:hi])
        bq_sb = cpool.tile([128, NPAIR], F32, tag="bq")
        bk_sb = cpool.tile([128, NPAIR], F32, tag="bk")
        bv_sb = cpool.tile([128, DG], F32, tag="bv")
        tri_sb = cpool.tile([128, 128], BF16, tag="tri")
        nc.scalar.dma_start(bq_sb[:], bq_d[:])
        nc.scalar.dma_start(bk_sb[:], bk_d[:])
        nc.scalar.dma_start(bv_sb[:], bv_d[:])
        nc.scalar.dma_start(tri_sb[:], tri_d[:])

        # chunk-0 inputs, split into pieces so the first projection fillers
        # unblock as soon as their ki-blocks land
        qin = inpool.tile([128, KI * SC], BF16, tag="qin", name="qin", bufs=3)
        for lo, hi in ((0, 1), (1, 2), (2, 4), (4, 6), (6, 8)):
            nc.gpsimd.dma_start(
                qin[:, lo * SC : hi * SC], qT_d[:, lo * SC : hi * SC]
            )
        nc.sync.dma_start(wk_sb[:, 1024:2048], wk_d[:, 1024:2048])
        kin = inpool.tile([128, KI * SC], BF16, tag="kin", name="kin", bufs=3)
        for (lo, hi), eng in (((0, 4), nc.sync), ((4, 8), nc.gpsimd)):
            eng.dma_start(
                kin[:, lo * SC : hi * SC], kT_d[:, lo * SC : hi * SC]
            )
        nc.sync.dma_start(wk_sb[:, 0:1024], wk_d[:, 0:1024])
        nc.gpsimd.dma_start(wv_sb[:], wv_d[:])
        # vin/wo ride the Act hardware-DGE queue: it is idle before the
        # first exps, giving a third parallel transfer queue at startup
        vin = stream_in(0, "vin", vT_d, nc.scalar)
        nc.scalar.dma_start(wo_sb[:], wo_d[:])

        # chunk-0: pair-1 Q/K projections drain now (attention needs them);
        # V + pair-0 projections become in-loop fillers.  V goes FIRST in
        # the hard queue: pair-1's AVs need V[0..3] right away, and they
        # must not sit behind chunk-1 projections that are gated on
        # chunk-1 input DMAs.
        fillers.extend(qk_proj_fillers(wq_sb, bq_sb, qin, 1, QT, 0))
        fillers.extend(qk_proj_fillers(wk_sb, bk_sb, kin, 1, KT, 0))
        drain(include_soft=False)
        for st in range(NST):
            fillers.extend(v_proj_fillers(wv_sb, bv_sb, vin, st, 0))
        fillers.extend(qk_proj_fillers(wq_sb, bq_sb, qin, 0, QT, 0))
        fillers.extend(qk_proj_fillers(wk_sb, bk_sb, kin, 0, KT, 0))
        if not causal:
            # full-width attention reads every chunk's K/V from chunk 0 on:
            # project everything up front (correctness over overlap)
            drain()
            for cc in range(1, NCH):
                qin, kin, vin = emit_input_dmas(cc)
                for t in (1, 0):
                    fillers.extend(
                        qk_proj_fillers(wq_sb, bq_sb, qin, t, QT, cc)
                    )
                    fillers.extend(
                        qk_proj_fillers(wk_sb, bk_sb, kin, t, KT, cc)
                    )
                for st in range(NST):
                    fillers.extend(v_proj_fillers(wv_sb, bv_sb, vin, st, cc))
                drain()

        # ---------- main loop over chunks ---------------------------------

        for c in range(NCH):
            # hard queue: QK-proj of c+1 (first-processed pair first);
            # soft queue: V-proj of c+1, then O-proj of c-1.
            # inputs are prefetched TWO chunks ahead (ring bufs=3) so the
            # c+1 projection fillers never stall the attention stream on
            # input arrival
            if causal and c == 0:
                INS[1] = emit_input_dmas(1)
                if NCH > 2:
                    INS[2] = emit_input_dmas(2)
            elif causal and c + 2 < NCH:
                INS[c + 2] = emit_input_dmas(c + 2)
            if causal and c + 1 < NCH:
                qin, kin, vin = INS[c + 1]
                for t in (1, 0):
                    fillers.extend(
                        qk_proj_fillers(wq_sb, bq_sb, qin, t, QT, c + 1)
                    )
                    fillers.extend(
                        qk_proj_fillers(wk_sb, bk_sb, kin, t, KT, c + 1)
                    )
                for st in range(NST):
                    soft.extend(v_proj_fillers(wv_sb, bv_sb, vin, st, c + 1))
            if c > 0:
                soft.extend(o_proj_fillers(c - 1))

            # attention j-loop for chunk c, per head pair
            for p in (1, 0):
                while ((p, c) not in QT or (p, c) not in KT) and (
                    fillers or soft
                ):
                    drain(1)
                njt = NST * (c + 1) if causal else NST * NCH
                ctx0 = psC.tile([65, SC], F32, tag="ctx")
                ctx1 = psC.tile([65, SC], F32, tag="ctx")
                h0, h1 = 2 * p, 2 * p + 1
                pending = None

                def emit_av(j, probs, first, last, q0):
                    nc.tensor.matmul(
                        ctx0[:, q0:SC],
                        V[j][:, 65 * h0 : 65 * h0 + 65],
                        probs[:, q0:SC],
                        start=first,
                        stop=last,
                        skip_group_check=True,
                    )
                    nc.tensor.matmul(
                        ctx1[:, q0:SC],
                        V[j][:, 65 * h1 : 65 * h1 + 65],
                        probs[:, SC + q0 : 2 * SC],
                        start=first,
                        stop=last,
                        skip_group_check=True,
                    )

                for j in range(njt):
                    jc, jt = divmod(j, NST)
                    diag = causal and jc == c
                    m = jt if diag else 0
                    q0 = 128 * m  # first valid query col in this chunk
                    scp = psS.tile([128, 2 * SC], F32, tag="sc")
                    nc.tensor.matmul(
                        scp[:, q0:SC],
                        KT[(p, jc)][0:64, jt * 128 : (jt + 1) * 128],
                        QT[(p, c)][0:64, q0:SC],
                        start=True,
                        stop=True,
                        tile_position=(0, 0),
                    )
                    nc.tensor.matmul(
                        scp[:, SC + q0 : 2 * SC],
                        KT[(p, jc)][64:128, jt * 128 : (jt + 1) * 128],
                        QT[(p, c)][64:128, q0:SC],
                        start=True,
                        stop=True,
                        tile_position=(64, 0),
                    )
                    probs = prpool.tile([128, 2 * SC], BF16, tag="probs")
                    if m == 0:
                        nc.scalar.activation(
                            probs[:], scp[:], AF.Exp, scale=0.125
                        )
                    else:
                        nc.scalar.activation(
                            probs[:, q0:SC], scp[:, q0:SC], AF.Exp, scale=0.125
                        )
                        nc.scalar.activation(
                            probs[:, SC + q0 : 2 * SC],
                            scp[:, SC + q0 : 2 * SC],
                            AF.Exp,
                            scale=0.125,
                        )
                    if diag:
                        # tri-mask the 128x128 diagonal block of both heads
                        for off in (0, SC):
                            lo = off + q0
                            nc.vector.tensor_mul(
                                probs[:, lo : lo + 128],
                                probs[:, lo : lo + 128],
                                tri_sb[:],
                            )
                    if pending is not None:
                        while pending[0] not in V and (fillers or soft):
                            drain(1)
                        emit_av(*pending)
                        drain(1)
                    pending = (j, probs, j == 0, j == njt - 1, q0)
                while pending[0] not in V and (fillers or soft):
                    drain(1)
                jL, probsL, firstL, lastL, q0L = pending
                nc.tensor.matmul(
                    ctx0[:, q0L:SC],
                    V[jL][:, 65 * h0 : 65 * h0 + 65],
                    probsL[:, q0L:SC],
                    start=firstL,
                    stop=lastL,
                    skip_group_check=True,
                )
                # ---- eager normalize v3: copy ctx psum -> SBUF raw f32
                # immediately (the copy is the ONLY psum reader, so psC
                # frees ~0.6us after the last AV instead of after the
                # whole recip->broadcast->mul chain, unblocking the next
                # pair's AVs).  Reciprocals use the ~5x faster approx
                # custom-DVE op.  The very last pair skips the copies and
                # normalizes straight from psum: nothing is waiting on the
                # psC banks there, and the tail wants the shortest chain.
                last_pair = p == 0 and c == NCH - 1
                rec = rcpool.tile([1, 2 * SC], F32, tag="recip")
                if last_pair:
                    raw0 = ctx0
                else:
                    raw0 = tmpool.tile([65, SC], F32, tag="raw0")
                    nc.vector.tensor_copy(raw0[:], ctx0[0:65, :])
                nc.vector.reciprocal_approx_fast(
                    rec[0:1, 0:SC], raw0[0:1, :]
                )
                nc.tensor.matmul(
                    ctx1[:, q0L:SC],
                    V[jL][:, 65 * h1 : 65 * h1 + 65],
                    probsL[:, SC + q0L : 2 * SC],
                    start=firstL,
                    stop=lastL,
                    skip_group_check=True,
                )
                if last_pair:
                    raw1 = ctx1
                else:
                    raw1 = tmpool.tile([65, SC], F32, tag="raw1")
                    nc.vector.tensor_copy(raw1[:], ctx1[0:65, :])
                nc.vector.reciprocal_approx_fast(
                    rec[0:1, SC : 2 * SC], raw1[0:1, :]
                )
                if p == 0:
                    # flush next-chunk projections now so their DVE
                    # bias-adds queue ahead of this pair's normalize muls
                    drain(include_soft=False)

                cx = cxpool.tile([128, SC], BF16, tag=f"cx{p}{c}")
                bc0 = bcpool.tile([65, SC], F32, tag="bc")
                nc.gpsimd.partition_broadcast(bc0[:], rec[0:1, 0:SC])
                bc1 = bcpool.tile([65, SC], F32, tag="bc")
                nc.gpsimd.partition_broadcast(bc1[:], rec[0:1, SC : 2 * SC])
                cxa = tmpool.tile([65, SC], BF16, tag="cxa")
                nc.vector.tensor_mul(cxa[0:65, :], raw0[0:65, :], bc0[0:65, :])
                tmp = tmpool.tile([65, SC], BF16, tag="tmp")
                nc.vector.tensor_mul(tmp[0:65, :], raw1[0:65, :], bc1[0:65, :])
                # cx merge DMAs both on sync (pure data waits there); the
                # very last pair's h1 merge goes on Act so the tail
                # O-projection isn't queued behind anything on sync
                h1eng = nc.scalar if last_pair else nc.sync
                nc.sync.dma_start(cx[0:64, :], cxa[1:65, :])
                h1eng.dma_start(cx[64:128, :], tmp[1:65, :])
                CX[(p, c)] = cx

            # boundary: flush hard queue (QK proj of c+1); soft carries over
            drain(include_soft=False)

        drain()
        o_proj_tail(NCH - 1)

    nc.finalize()
    return nc


def get_program(causal: bool):
    if causal not in _programs:
        _programs[causal] = _build_program(causal)
    return _programs[causal]


def _tile_seq(xT):
    """[D, S] -> [128, NCH*KI*SC]: chunk-major, then 128-row d-block (ki)."""
    return np.ascontiguousarray(
        xT.reshape(KI, 128, NCH, SC).transpose(1, 2, 0, 3).reshape(128, -1)
    )


def _tile_w(wT, nblk):
    """[nblk*128, M] -> [128, nblk*M]: 128-row block i at cols [i*M,(i+1)*M)."""
    m = wT.shape[1]
    return np.ascontiguousarray(
        wT.reshape(nblk, 128, m).transpose(1, 0, 2).reshape(128, nblk * m)
    )


def _tile_w_tmaj(wT):
    """[D, DG] -> [128, 2*KI*128], t-major: (t, ki) block at
    cols [(t*KI+ki)*128, ...)."""
    return np.ascontiguousarray(
        wT.reshape(KI, 128, NPAIR, 128)
        .transpose(1, 2, 0, 3)
        .reshape(128, NPAIR * KI * 128)
    )


def _make_core_inputs(query, key, value, wq, bq, wk, bk, wv, bv, wo):
    import ml_dtypes

    bf16 = ml_dtypes.bfloat16
    f32 = np.float32
    tri = np.triu(np.ones((128, 128), f32)).astype(bf16)
    in_maps = []
    qTt = [_tile_seq(query[b].T.astype(bf16)) for b in range(B)]
    kTt = [_tile_seq(key[b].T.astype(bf16)) for b in range(B)]
    vTt = [_tile_seq(value[b].T.astype(bf16)) for b in range(B)]
    for core in range(NCORES):
        b, g = divmod(core, G)
        sl = slice(g * DG, (g + 1) * DG)
        in_maps.append(
            {
                "qT": qTt[b],
                "kT": kTt[b],
                "vT": vTt[b],
                "wqT": _tile_w_tmaj(wq[sl, :].T.astype(bf16)),
                "wkT": _tile_w_tmaj(wk[sl, :].T.astype(bf16)),
                "wvT": _tile_w(wv[sl, :].T.astype(bf16), KI),
                "woT": _tile_w(wo[:, sl].T.astype(bf16), 2),
                "bq2": np.ascontiguousarray(bq[sl].reshape(NPAIR, 128).T, f32),
                "bk2": np.ascontiguousarray(bk[sl].reshape(NPAIR, 128).T, f32),
                "bvb": np.ascontiguousarray(
                    np.broadcast_to(bv[sl], (128, DG)), f32
                ),
                "tri": tri,
            }
        )
    return in_maps


def _numpy_fallback(query, key, value, mask, wq, bq, wk, bk, wv, bv, wo, bo):
    out = np.empty((B, S, D), np.float32)
    for b in range(B):
        Q = (query[b] @ wq.T + bq).reshape(S, H, DK).transpose(1, 0, 2)
        K = (key[b] @ wk.T + bk).reshape(S, H, DK).transpose(1, 0, 2)
        Vv = (value[b] @ wv.T + bv).reshape(S, H, DK).transpose(1, 0, 2)
        sc = np.einsum("hqd,hkd->hqk", Q, K) / np.sqrt(np.float32(DK))
        sc = np.where(mask[b][None] == 0, -np.inf, sc)
        sc = sc - sc.max(axis=-1, keepdims=True)
        e = np.exp(sc)
        attn = e / e.sum(axis=-1, keepdims=True)
        ctx = np.einsum("hqk,hkd->hqd", attn, Vv)
        out[b] = ctx.transpose(1, 0, 2).reshape(S, D) @ wo.T + bo
    return out


def kernel(query, key, value, mask, wq, bq, wk, bk, wv, bv, wo, bo):
    global LAST_RESULT
    query = np.asarray(query, np.float32)
    key = np.asarray(key, np.float32)
    value = np.asarray(value, np.float32)
    mask = np.asarray(mask)
    wq, bq = np.asarray(wq, np.float32), np.asarray(bq, np.float32)
    wk, bk = np.asarray(wk, np.float32), np.asarray(bk, np.float32)
    wv, bv = np.asarray(wv, np.float32), np.asarray(bv, np.float32)
    wo, bo = np.asarray(wo, np.float32), np.asarray(bo, np.float32)

    tril = np.tril(np.ones((S, S), mask.dtype))
    if all((mask[b] == tril).all() for b in range(B)):
        causal = True
    elif (mask == 1).all():
        causal = False
    else:
        return _numpy_fallback(
            query, key, value, mask, wq, bq, wk, bk, wv, bv, wo, bo
        )

    from concourse.bass_utils import run_bass_kernel_spmd

    nc = get_program(causal)
    in_maps = _make_core_inputs(query, key, value, wq, bq, wk, bk, wv, bv, wo)
    trace = bool(int(os.environ.get("MHA_TRACE", "0")))
    res = run_bass_kernel_spmd(nc, in_maps, list(range(NCORES)), trace=trace)
    LAST_RESULT = res

    out = np.zeros((B, S, D), np.float32)
    for core in range(NCORES):
        b = core // G
        out[b] += np.asarray(res.results[core]["out"], np.float32)
    out += bo[None, None, :]
    return out



# revision 35
# speedup vs baseline: 1.0301x; 1.0124x over previous
"""MultiHeadAttention forward on 8 Trainium2 NeuronCores (Bass/Tile), v2.

Problem (hardcoded): B=2, S=2048, D=1024, H=16, DK=64, causal mask.

Sharding: data-parallel over batch (2) x tensor-parallel over heads
(4 heads per core).  core = 4*b + g handles batch b, heads [4g, 4g+4).
Host sums the 4 partial O outputs per batch and adds bo.

v2 restructure vs baseline:
  - Global software pipeline: next-chunk QKV-projection and prev-chunk
    O-projection matmuls are emitted as "filler" units interleaved into
    the attention j-loop, so the PE never starves while the Activation
    engine paces exp().
  - Causal handling: scores/exp/AV for diagonal k-tiles are
    range-restricted to valid queries; only the 128x128 diagonal block
    gets a tri-mask multiply (no DVE column zeroing).
  - Normalize chain: partition_broadcast reads the reciprocal row at
    partition 64 directly (no 1-partition staging DMA).
  - QT/KT in bf16; output partials in bf16 (host sums in f32).
  - DMA spread: wq/wk + k input on SP queue, q/v inputs + wv/wo + out
    on Pool queue, biases/tri/wo_lo on Act queue.
  - PSUM: sc 2x[128,1024] (4 banks) + ctx 2x[65,512] (2) + wk 2x[128,512]
    (2) = 8 banks.
  - Tail: last chunk's O-projection split into waves (pair-1 K=128, h0
    K=64 from cx, h1 K=64 from tmp via lane-shifted wo_lo) so no cx
    merge DMA sits on the critical path.
"""

import os
import sys

sys.path.insert(0, "/opt/trn_rl_repo")

import numpy as np

B, S, D, H = 2, 2048, 1024, 16
DK = D // H          # 64
NCORES = 8
G = 4                # heads per core
DG = G * DK          # 256 output dims per core
SC = 512             # seq chunk
NCH = S // SC        # 4
KI = D // 128        # 8 contraction chunks
NPAIR = 2            # head pairs per core
NST = SC // 128      # 4 seq 128-tiles per chunk

_programs = {}
LAST_RESULT = None


def _build_program(causal: bool, reps: int = 1):
    import concourse.bass as bass
    import concourse.tile as tile
    import concourse.mybir as mybir
    from concourse import bacc
    from contextlib import ExitStack
    from collections import deque

    F32 = mybir.dt.float32
    F32R = mybir.dt.float32r
    BF16 = mybir.dt.bfloat16
    AF = mybir.ActivationFunctionType

    nc = bacc.Bacc()
    # inputs are PRE-TILED host-side into their exact SBUF layouts so every
    # load is a flat contiguous DMA (128 descriptors x 4-8KB, no gather):
    #   qT/kT/vT: [128, NCH*8*SC]; chunk c occupies cols [c*8*SC,(c+1)*8*SC)
    #   with the ki-th 128-row d-block of that chunk at sub-cols
    #   [ki*SC,(ki+1)*SC) (h-major: ki = 4*h + a).
    qT_d = nc.dram_tensor("qT", [128, NCH * KI * SC], BF16, kind="ExternalInput")
    kT_d = nc.dram_tensor("kT", [128, NCH * KI * SC], BF16, kind="ExternalInput")
    vT_d = nc.dram_tensor("vT", [128, NCH * KI * SC], BF16, kind="ExternalInput")
    wq_d = nc.dram_tensor("wqT", [128, KI * DG], BF16, kind="ExternalInput")
    wk_d = nc.dram_tensor("wkT", [128, KI * DG], BF16, kind="ExternalInput")
    wv_d = nc.dram_tensor("wvT", [128, KI * DG], BF16, kind="ExternalInput")
    wo_d = nc.dram_tensor("woT", [128, 2 * D], BF16, kind="ExternalInput")
    bq_d = nc.dram_tensor("bq2", [128, NPAIR], F32, kind="ExternalInput")
    bk_d = nc.dram_tensor("bk2", [128, NPAIR], F32, kind="ExternalInput")
    bv_d = nc.dram_tensor("bvb", [128, DG], F32, kind="ExternalInput")
    tri_d = nc.dram_tensor("tri", [128, 128], BF16, kind="ExternalInput")
    out_d = nc.dram_tensor("out", [S, D], BF16, kind="ExternalOutput")

    with ExitStack() as ctx:
        tc = ctx.enter_context(tile.TileContext(nc))
        cpool = ctx.enter_context(tc.tile_pool(name="const", bufs=1))
        inpool = ctx.enter_context(tc.tile_pool(name="ins", bufs=5))
        qtpool = ctx.enter_context(tc.tile_pool(name="qt", bufs=1))
        ktpool = ctx.enter_context(tc.tile_pool(name="kt", bufs=1))
        vpool = ctx.enter_context(tc.tile_pool(name="v", bufs=1))
        cxpool = ctx.enter_context(tc.tile_pool(name="cx", bufs=1))
        prpool = ctx.enter_context(tc.tile_pool(name="probs", bufs=4))
        rcpool = ctx.enter_context(tc.tile_pool(name="recip", bufs=3))
        bcpool = ctx.enter_context(tc.tile_pool(name="bc", bufs=4))
        tmpool = ctx.enter_context(tc.tile_pool(name="tmp", bufs=3))
        outpool = ctx.enter_context(tc.tile_pool(name="osb", bufs=6))
        psS = ctx.enter_context(tc.tile_pool(name="psS", bufs=2, space="PSUM"))
        psC = ctx.enter_context(tc.tile_pool(name="psC", bufs=2, space="PSUM"))
        psW = ctx.enter_context(tc.tile_pool(name="psW", bufs=2, space="PSUM"))

        QT = {}
        KT = {}
        V = {}
        CX = {}
        INS = {}

        fillers = deque()   # hard: QK-proj of next chunk; flushed at boundary
        soft = deque()      # V-proj of next chunk + O-proj of prev chunk

        def drain(n=None, include_soft=True):
            k = (len(fillers) + (len(soft) if include_soft else 0)) if n is None else n
            for _ in range(k):
                if fillers:
                    fillers.popleft()()
                elif include_soft and soft:
                    soft.popleft()()
                else:
                    break

        # ---------- filler-unit builders (each closure = ~1 PE matmul) ----

        def qk_proj_fillers(w_sb, b_sb, in_sb, t, dst, c):
            """8 matmuls accumulating one [128,512] psum + bias-add -> bf16."""
            state = {}

            def mk(ki):
                def f():
                    if ki == 0:
                        state["ps"] = psW.tile(
                            [128, SC], F32, tag="wk", name="wkps"
                        )
                    nc.tensor.matmul(
                        state["ps"][:],
                        w_sb[:, (t * KI + ki) * 128 : (t * KI + ki + 1) * 128],
                        in_sb[:, ki * SC : (ki + 1) * SC],
                        start=(ki == 0),
                        stop=(ki == KI - 1),
                    )
                    if ki == KI - 1:
                        pool = qtpool if dst is QT else ktpool
                        tl = pool.tile(
                            [128, SC], BF16,
                            tag=f"{'q' if dst is QT else 'k'}{t}{c}",
                            name="qkt",
                        )
                        nc.vector.tensor_scalar_add(
                            tl[:], state["ps"][:], b_sb[:, t : t + 1]
                        )
                        dst[(t, c)] = tl

                return f

            return [mk(ki) for ki in range(KI)]

        def v_proj_fillers(wv_sb, bv_sb, in_sb, st, c):
            """8 matmuls -> [128,256] psum + assemble V[j] with ones cols."""
            j = c * NST + st
            state = {}

            def mk(ki):
                def f():
                    if ki == 0:
                        state["ps"] = psW.tile(
                            [128, SC], F32, tag="wk", name="wkps"
                        )
                    nc.tensor.matmul(
                        state["ps"][:, 0:DG],
                        in_sb[
                            :,
                            ki * SC + st * 128 : ki * SC + (st + 1) * 128,
                        ],
                        wv_sb[:, ki * DG : (ki + 1) * DG],
                        start=(ki == 0),
                        stop=(ki == KI - 1),
                    )
                    if ki == KI - 1:
                        vt = vpool.tile(
                            [128, G * 65], BF16, tag=f"v{j}", name="vt"
                        )
                        nc.vector.tensor_scalar(
                            vt[:].rearrange("p (h x) -> p h x", x=65)[:, :, 0:1],
                            bv_sb[:].rearrange("p (a b) -> p a b", b=1)[:, 0:G, :],
                            0.0,
                            1.0,
                            mybir.AluOpType.mult,
                            mybir.AluOpType.add,
                        )
                        nc.vector.tensor_add(
                            vt[:].rearrange("p (h x) -> p h x", x=65)[:, :, 1:65],
                            state["ps"][:, 0:DG].rearrange(
                                "p (h x) -> p h x", x=64
                            ),
                            bv_sb[:].rearrange("p (h x) -> p h x", x=64),
                        )
                        V[j] = vt

                return f

            return [mk(ki) for ki in range(KI)]

        def o_proj_fillers(c):
            """Partial O-projection for chunk c: per (st, half) one 2-matmul
            unit + psum->osb bf16 copy; DMA per st row."""
            units = []
            state = {}

            def mk(st, n):
                def f():
                    if n == 0:
                        state[st] = outpool.tile(
                            [128, D], BF16, tag="osb", name="osb"
                        )
                    ps = psW.tile([128, SC], F32, tag="wk", name="wkps")
                    for p in (1, 0):
                        nc.tensor.matmul(
                            ps[:],
                            CX[(p, c)][:, st * 128 : (st + 1) * 128],
                            wo_sb[:, p * D + n * SC : p * D + (n + 1) * SC],
                            start=(p == 1),
                            stop=(p == 0),
                        )
                    osb = state[st]
                    if n == 0:
                        nc.vector.tensor_copy(osb[:, 0:SC], ps[:])
                    else:
                        nc.vector.tensor_copy(osb[:, SC : 2 * SC], ps[:])
                        r0 = (c * NST + st) * 128
                        # sync queue, not gpsimd: keeps the gpsimd queue
                        # free for the latency-critical partition_broadcasts
                        nc.sync.dma_start(out_d[r0 : r0 + 128, :], osb[:])

                return f

            for st in range(NST):
                for n in range(2):
                    units.append(mk(st, n))
            return units

        def o_proj_tail(c):
            """Tail O-projection: pair-1 contractions first (its cx merges
            early), then pair-0 contractions + copies/DMAs spread over
            DVE/Act + SP/Pool/Act queues.  psum spread over psW/psC/psS."""
            pss = {}
            for n in range(2):
                pss[(0, n)] = psW.tile([128, SC], F32, tag="wk", name="otps")
                pss[(1, n)] = psC.tile([128, SC], F32, tag="ctx", name="otps")
            for st in (2, 3):
                big = psS.tile([128, 2 * SC], F32, tag="sc", name="otps")
                pss[(st, 0)] = big[:, 0:SC]
                pss[(st, 1)] = big[:, SC : 2 * SC]
            # wave A: pair-1 contractions for sts not on psC (ready early)
            for st in (0, 2, 3):
                for n in range(2):
                    nc.tensor.matmul(
                        pss[(st, n)],
                        CX[(1, c)][:, st * 128 : (st + 1) * 128],
                        wo_sb[:, D + n * SC : D + (n + 1) * SC],
                        start=True,
                        stop=False,
                        skip_group_check=True,
                    )
            # st1 on psC slots (free once the normalize consumed ctx0/ctx1)
            for n in range(2):
                nc.tensor.matmul(
                    pss[(1, n)],
                    CX[(1, c)][:, 128:256],
                    wo_sb[:, D + n * SC : D + (n + 1) * SC],
                    start=True,
                    stop=False,
                    skip_group_check=True,
                )
            # wave B: pair-0 contractions on merged cx + copies (DVE/Act
            # alternating) + bf16 out DMAs on sync/gpsimd
            ei = 0
            copies = (nc.vector, nc.scalar, nc.vector, nc.scalar,
                      nc.vector, nc.scalar, nc.vector, nc.scalar)
            dmas = (nc.sync, nc.gpsimd, nc.sync, nc.gpsimd,
                    nc.sync, nc.gpsimd, nc.sync, nc.gpsimd)
            osbs = {}
            for st in (0, 2, 3, 1):
                osbs[st] = outpool.tile([128, D], BF16, tag="osb", name="osb")
            for st in (0, 2, 3, 1):
                osb = osbs[st]
                r0 = (c * NST + st) * 128
                for n in range(2):
                    nc.tensor.matmul(
                        pss[(st, n)],
                        CX[(0, c)][:, st * 128 : (st + 1) * 128],
                        wo_sb[:, n * SC : (n + 1) * SC],
                        start=False,
                        stop=True,
                        skip_group_check=True,
                    )
                    ceng = copies[ei]
                    if ceng is nc.scalar:
                        ceng.copy(osb[:, n * SC : (n + 1) * SC], pss[(st, n)])
                    else:
                        ceng.tensor_copy(
                            osb[:, n * SC : (n + 1) * SC], pss[(st, n)]
                        )
                    dmas[ei].dma_start(
                        out_d[r0 : r0 + 128, n * SC : (n + 1) * SC],
                        osb[:, n * SC : (n + 1) * SC],
                    )
                    ei += 1

        def stream_in(c, tag, t_d, eng):
            t_sb = inpool.tile(
                [128, KI * SC], BF16, tag=tag, name="t_sb", bufs=3
            )
            eng.dma_start(
                t_sb[:], t_d[:, c * KI * SC : (c + 1) * KI * SC]
            )
            return t_sb

        def emit_input_dmas(c):
            # all on sync: a gpsimd dma_start unloads the
            # partition_broadcast library (MODIFY_POOL_CONFIG) and the
            # reload waits on the gpsimd DMA queue drain — never mix them
            qin = stream_in(c, "qin", qT_d, nc.sync)
            kin = stream_in(c, "kin", kT_d, nc.sync)
            vin = stream_in(c, "vin", vT_d, nc.sync)
            return qin, kin, vin

        # ---------- startup -------------------------------------------------

        if reps > 1:
            ctx.enter_context(
                tc.For_i(
                    0,
                    reps,
                    1,
                    hint_engines=(
                        mybir.EngineType.PE,
                        mybir.EngineType.Activation,
                        mybir.EngineType.DVE,
                        mybir.EngineType.SP,
                        mybir.EngineType.Pool,
                    ),
                )
            )

        wq_sb = cpool.tile([128, KI * DG], BF16, tag="wq")
        wk_sb = cpool.tile([128, KI * DG], BF16, tag="wk")
        wv_sb = cpool.tile([128, KI * DG], BF16, tag="wv")
        wo_sb = cpool.tile([128, 2 * D], BF16, tag="wo")
        # wq/wk are t-major host-side; pair t=1 is projected first, so its
        # half loads first
        for lo, hi in ((1024, 1536), (1536, 2048), (0, 1024)):
            nc.sync.dma_start(wq_sb[:, lo:hi], wq_d[:, lo:hi])
        bq_sb = cpool.tile([128, NPAIR], F32, tag="bq")
        bk_sb = cpool.tile([128, NPAIR], F32, tag="bk")
        bv_sb = cpool.tile([128, DG], F32, tag="bv")
        tri_sb = cpool.tile([128, 128], BF16, tag="tri")
        nc.scalar.dma_start(bq_sb[:], bq_d[:])
        nc.scalar.dma_start(bk_sb[:], bk_d[:])
        nc.scalar.dma_start(bv_sb[:], bv_d[:])
        nc.scalar.dma_start(tri_sb[:], tri_d[:])

        # chunk-0 inputs, split into pieces so the first projection fillers
        # unblock as soon as their ki-blocks land; spread across the
        # gpsimd and Act queues so both halves stream in parallel
        qin = inpool.tile([128, KI * SC], BF16, tag="qin", name="qin", bufs=3)
        for (lo, hi), eng in (
            ((0, 1), nc.gpsimd),
            ((1, 2), nc.gpsimd),
            ((2, 4), nc.gpsimd),
            ((4, 6), nc.scalar),
            ((6, 8), nc.scalar),
        ):
            eng.dma_start(
                qin[:, lo * SC : hi * SC], qT_d[:, lo * SC : hi * SC]
            )
        nc.sync.dma_start(wk_sb[:, 1024:2048], wk_d[:, 1024:2048])
        kin = inpool.tile([128, KI * SC], BF16, tag="kin", name="kin", bufs=3)
        for (lo, hi), eng in (((0, 4), nc.sync), ((4, 8), nc.gpsimd)):
            eng.dma_start(
                kin[:, lo * SC : hi * SC], kT_d[:, lo * SC : hi * SC]
            )
        nc.sync.dma_start(wk_sb[:, 0:1024], wk_d[:, 0:1024])
        nc.gpsimd.dma_start(wv_sb[:], wv_d[:])
        # vin/wo ride the Act hardware-DGE queue: it is idle before the
        # first exps, giving a third parallel transfer queue at startup
        vin = stream_in(0, "vin", vT_d, nc.scalar)
        nc.scalar.dma_start(wo_sb[:], wo_d[:])

        # chunk-0: pair-1 Q/K projections drain now (attention needs them);
        # V + pair-0 projections become in-loop fillers.  V goes FIRST in
        # the hard queue: pair-1's AVs need V[0..3] right away, and they
        # must not sit behind chunk-1 projections that are gated on
        # chunk-1 input DMAs.
        fillers.extend(qk_proj_fillers(wq_sb, bq_sb, qin, 1, QT, 0))
        fillers.extend(qk_proj_fillers(wk_sb, bk_sb, kin, 1, KT, 0))
        drain(include_soft=False)
        for st in range(NST):
            fillers.extend(v_proj_fillers(wv_sb, bv_sb, vin, st, 0))
        fillers.extend(qk_proj_fillers(wq_sb, bq_sb, qin, 0, QT, 0))
        fillers.extend(qk_proj_fillers(wk_sb, bk_sb, kin, 0, KT, 0))
        if not causal:
            # full-width attention reads every chunk's K/V from chunk 0 on:
            # project everything up front (correctness over overlap)
            drain()
            for cc in range(1, NCH):
                qin, kin, vin = emit_input_dmas(cc)
                for t in (1, 0):
                    fillers.extend(
                        qk_proj_fillers(wq_sb, bq_sb, qin, t, QT, cc)
                    )
                    fillers.extend(
                        qk_proj_fillers(wk_sb, bk_sb, kin, t, KT, cc)
                    )
                for st in range(NST):
                    fillers.extend(v_proj_fillers(wv_sb, bv_sb, vin, st, cc))
                drain()

        # ---------- main loop over chunks ---------------------------------

        for c in range(NCH):
            # hard queue: QK-proj of c+1 (first-processed pair first);
            # soft queue: V-proj of c+1, then O-proj of c-1.
            # inputs are prefetched TWO chunks ahead (ring bufs=3) so the
            # c+1 projection fillers never stall the attention stream on
            # input arrival
            if causal and c == 0:
                INS[1] = emit_input_dmas(1)
                if NCH > 2:
                    INS[2] = emit_input_dmas(2)
            elif causal and c + 2 < NCH:
                INS[c + 2] = emit_input_dmas(c + 2)
            if causal and c + 1 < NCH:
                qin, kin, vin = INS[c + 1]
                for t in (1, 0):
                    fillers.extend(
                        qk_proj_fillers(wq_sb, bq_sb, qin, t, QT, c + 1)
                    )
                    fillers.extend(
                        qk_proj_fillers(wk_sb, bk_sb, kin, t, KT, c + 1)
                    )
                for st in range(NST):
                    soft.extend(v_proj_fillers(wv_sb, bv_sb, vin, st, c + 1))
            if c > 0:
                soft.extend(o_proj_fillers(c - 1))

            # attention j-loop for chunk c, per head pair
            for p in (1, 0):
                while ((p, c) not in QT or (p, c) not in KT) and (
                    fillers or soft
                ):
                    drain(1)
                njt = NST * (c + 1) if causal else NST * NCH
                ctx0 = psC.tile([65, SC], F32, tag="ctx")
                ctx1 = psC.tile([65, SC], F32, tag="ctx")
                h0, h1 = 2 * p, 2 * p + 1
                pending = None

                def emit_av(j, probs, first, last, q0):
                    nc.tensor.matmul(
                        ctx0[:, q0:SC],
                        V[j][:, 65 * h0 : 65 * h0 + 65],
                        probs[:, q0:SC],
                        start=first,
                        stop=last,
                        skip_group_check=True,
                    )
                    nc.tensor.matmul(
                        ctx1[:, q0:SC],
                        V[j][:, 65 * h1 : 65 * h1 + 65],
                        probs[:, SC + q0 : 2 * SC],
                        start=first,
                        stop=last,
                        skip_group_check=True,
                    )

                for j in range(njt):
                    jc, jt = divmod(j, NST)
                    diag = causal and jc == c
                    m = jt if diag else 0
                    q0 = 128 * m  # first valid query col in this chunk
                    scp = psS.tile([128, 2 * SC], F32, tag="sc")
                    nc.tensor.matmul(
                        scp[:, q0:SC],
                        KT[(p, jc)][0:64, jt * 128 : (jt + 1) * 128],
                        QT[(p, c)][0:64, q0:SC],
                        start=True,
                        stop=True,
                        tile_position=(0, 0),
                    )
                    nc.tensor.matmul(
                        scp[:, SC + q0 : 2 * SC],
                        KT[(p, jc)][64:128, jt * 128 : (jt + 1) * 128],
                        QT[(p, c)][64:128, q0:SC],
                        start=True,
                        stop=True,
                        tile_position=(64, 0),
                    )
                    probs = prpool.tile([128, 2 * SC], BF16, tag="probs")
                    if m == 0:
                        nc.scalar.activation(
                            probs[:], scp[:], AF.Exp, scale=0.125
                        )
                    else:
                        nc.scalar.activation(
                            probs[:, q0:SC], scp[:, q0:SC], AF.Exp, scale=0.125
                        )
                        nc.scalar.activation(
                            probs[:, SC + q0 : 2 * SC],
                            scp[:, SC + q0 : 2 * SC],
                            AF.Exp,
                            scale=0.125,
                        )
                    if diag:
                        # tri-mask the 128x128 diagonal block of both heads
                        for off in (0, SC):
                            lo = off + q0
                            nc.vector.tensor_mul(
                                probs[:, lo : lo + 128],
                                probs[:, lo : lo + 128],
                                tri_sb[:],
                            )
                    if pending is not None:
                        while pending[0] not in V and (fillers or soft):
                            drain(1)
                        emit_av(*pending)
                        drain(1)
                    pending = (j, probs, j == 0, j == njt - 1, q0)
                while pending[0] not in V and (fillers or soft):
                    drain(1)
                jL, probsL, firstL, lastL, q0L = pending
                nc.tensor.matmul(
                    ctx0[:, q0L:SC],
                    V[jL][:, 65 * h0 : 65 * h0 + 65],
                    probsL[:, q0L:SC],
                    start=firstL,
                    stop=lastL,
                    skip_group_check=True,
                )
                # ---- eager normalize v3: copy ctx psum -> SBUF raw f32
                # immediately (the copy is the ONLY psum reader, so psC
                # frees ~0.6us after the last AV instead of after the
                # whole recip->broadcast->mul chain, unblocking the next
                # pair's AVs).  Reciprocals use the ~5x faster approx
                # custom-DVE op.  The very last pair skips the copies and
                # normalizes straight from psum: nothing is waiting on the
                # psC banks there, and the tail wants the shortest chain.
                last_pair = p == 0 and c == NCH - 1
                rec = rcpool.tile([1, 2 * SC], F32, tag="recip")
                if last_pair:
                    raw0 = ctx0
                else:
                    raw0 = tmpool.tile([65, SC], F32, tag="raw0")
                    nc.vector.tensor_copy(raw0[:], ctx0[0:65, :])
                nc.vector.reciprocal_approx_fast(
                    rec[0:1, 0:SC], raw0[0:1, :]
                )
                nc.tensor.matmul(
                    ctx1[:, q0L:SC],
                    V[jL][:, 65 * h1 : 65 * h1 + 65],
                    probsL[:, SC + q0L : 2 * SC],
                    start=firstL,
                    stop=lastL,
                    skip_group_check=True,
                )
                if last_pair:
                    raw1 = ctx1
                else:
                    raw1 = tmpool.tile([65, SC], F32, tag="raw1")
                    nc.vector.tensor_copy(raw1[:], ctx1[0:65, :])
                nc.vector.reciprocal_approx_fast(
                    rec[0:1, SC : 2 * SC], raw1[0:1, :]
                )
                if p == 0:
                    # flush next-chunk projections now so their DVE
                    # bias-adds queue ahead of this pair's normalize muls
                    drain(include_soft=False)

                cx = cxpool.tile([128, SC], BF16, tag=f"cx{p}{c}")
                bc0 = bcpool.tile([65, SC], F32, tag="bc")
                nc.gpsimd.partition_broadcast(bc0[:], rec[0:1, 0:SC])
                bc1 = bcpool.tile([65, SC], F32, tag="bc")
                nc.gpsimd.partition_broadcast(bc1[:], rec[0:1, SC : 2 * SC])
                cxa = tmpool.tile([65, SC], BF16, tag="cxa")
                nc.vector.tensor_mul(cxa[0:65, :], raw0[0:65, :], bc0[0:65, :])
                tmp = tmpool.tile([65, SC], BF16, tag="tmp")
                nc.vector.tensor_mul(tmp[0:65, :], raw1[0:65, :], bc1[0:65, :])
                # cx merge DMAs both on sync (pure data waits there); the
                # very last pair's h1 merge goes on Act so the tail
                # O-projection isn't queued behind anything on sync
                h1eng = nc.scalar if last_pair else nc.sync
                nc.sync.dma_start(cx[0:64, :], cxa[1:65, :])
                h1eng.dma_start(cx[64:128, :], tmp[1:65, :])
                CX[(p, c)] = cx

            # boundary: flush hard queue (QK proj of c+1); soft carries over
            drain(include_soft=False)

        drain()
        o_proj_tail(NCH - 1)

    nc.finalize()
    return nc


def get_program(causal: bool):
    if causal not in _programs:
        _programs[causal] = _build_program(causal)
    return _programs[causal]


def _tile_seq(xT):
    """[D, S] -> [128, NCH*KI*SC]: chunk-major, then 128-row d-block (ki)."""
    return np.ascontiguousarray(
        xT.reshape(KI, 128, NCH, SC).transpose(1, 2, 0, 3).reshape(128, -1)
    )


def _tile_w(wT, nblk):
    """[nblk*128, M] -> [128, nblk*M]: 128-row block i at cols [i*M,(i+1)*M)."""
    m = wT.shape[1]
    return np.ascontiguousarray(
        wT.reshape(nblk, 128, m).transpose(1, 0, 2).reshape(128, nblk * m)
    )


def _tile_w_tmaj(wT):
    """[D, DG] -> [128, 2*KI*128], t-major: (t, ki) block at
    cols [(t*KI+ki)*128, ...)."""
    return np.ascontiguousarray(
        wT.reshape(KI, 128, NPAIR, 128)
        .transpose(1, 2, 0, 3)
        .reshape(128, NPAIR * KI * 128)
    )


def _make_core_inputs(query, key, value, wq, bq, wk, bk, wv, bv, wo):
    import ml_dtypes

    bf16 = ml_dtypes.bfloat16
    f32 = np.float32
    tri = np.triu(np.ones((128, 128), f32)).astype(bf16)
    in_maps = []
    qTt = [_tile_seq(query[b].T.astype(bf16)) for b in range(B)]
    kTt = [_tile_seq(key[b].T.astype(bf16)) for b in range(B)]
    vTt = [_tile_seq(value[b].T.astype(bf16)) for b in range(B)]
    for core in range(NCORES):
        b, g = divmod(core, G)
        sl = slice(g * DG, (g + 1) * DG)
        in_maps.append(
            {
                "qT": qTt[b],
                "kT": kTt[b],
                "vT": vTt[b],
                "wqT": _tile_w_tmaj(wq[sl, :].T.astype(bf16)),
                "wkT": _tile_w_tmaj(wk[sl, :].T.astype(bf16)),
                "wvT": _tile_w(wv[sl, :].T.astype(bf16), KI),
                "woT": _tile_w(wo[:, sl].T.astype(bf16), 2),
                "bq2": np.ascontiguousarray(bq[sl].reshape(NPAIR, 128).T, f32),
                "bk2": np.ascontiguousarray(bk[sl].reshape(NPAIR, 128).T, f32),
                "bvb": np.ascontiguousarray(
                    np.broadcast_to(bv[sl], (128, DG)), f32
                ),
                "tri": tri,
            }
        )
    return in_maps


def _numpy_fallback(query, key, value, mask, wq, bq, wk, bk, wv, bv, wo, bo):
    out = np.empty((B, S, D), np.float32)
    for b in range(B):
        Q = (query[b] @ wq.T + bq).reshape(S, H, DK).transpose(1, 0, 2)
        K = (key[b] @ wk.T + bk).reshape(S, H, DK).transpose(1, 0, 2)
        Vv = (value[b] @ wv.T + bv).reshape(S, H, DK).transpose(1, 0, 2)
        sc = np.einsum("hqd,hkd->hqk", Q, K) / np.sqrt(np.float32(DK))
        sc = np.where(mask[b][None] == 0, -np.inf, sc)
        sc = sc - sc.max(axis=-1, keepdims=True)
        e = np.exp(sc)
        attn = e / e.sum(axis=-1, keepdims=True)
        ctx = np.einsum("hqk,hkd->hqd", attn, Vv)
        out[b] = ctx.transpose(1, 0, 2).reshape(S, D) @ wo.T + bo
    return out


def kernel(query, key, value, mask, wq, bq, wk, bk, wv, bv, wo, bo):
    global LAST_RESULT
    query = np.asarray(query, np.float32)
    key = np.asarray(key, np.float32)
    value = np.asarray(value, np.float32)
    mask = np.asarray(mask)
    wq, bq = np.asarray(wq, np.float32), np.asarray(bq, np.float32)
    wk, bk = np.asarray(wk, np.float32), np.asarray(bk, np.float32)
    wv, bv = np.asarray(wv, np.float32), np.asarray(bv, np.float32)
    wo, bo = np.asarray(wo, np.float32), np.asarray(bo, np.float32)

    tril = np.tril(np.ones((S, S), mask.dtype))
    if all((mask[b] == tril).all() for b in range(B)):
        causal = True
    elif (mask == 1).all():
        causal = False
    else:
        return _numpy_fallback(
            query, key, value, mask, wq, bq, wk, bk, wv, bv, wo, bo
        )

    from concourse.bass_utils import run_bass_kernel_spmd

    nc = get_program(causal)
    in_maps = _make_core_inputs(query, key, value, wq, bq, wk, bk, wv, bv, wo)
    trace = bool(int(os.environ.get("MHA_TRACE", "0")))
    res = run_bass_kernel_spmd(nc, in_maps, list(range(NCORES)), trace=trace)
    LAST_RESULT = res

    out = np.zeros((B, S, D), np.float32)
    for core in range(NCORES):
        b = core // G
        out[b] += np.asarray(res.results[core]["out"], np.float32)
    out += bo[None, None, :]
    return out

